# revision 51
# baseline (speedup 1.0000x reference)
"""Trainium2 Bass kernel for nn_DDPMVAEQueryEncoder.

Strategy (data-parallel over batch, 8 cores):
  * Host: bucket/pack rows into 4 bands of 1024 (fattest band first) to
    minimize gather padding; build int16 gather-index tiles; fold all
    weight-only matmuls; fold timestep embeddings into the x-state
    (x~ = x + temb_t) with per-step corrections folded into the noise
    tensor; precompute 1/sqrt(nnz) per row.
  * Device per core (512 batch rows):
      phase 1: bf16 embedding table with 256B row pitch gathered via
        128B-element dma_gather (one descriptor per lookup at half the
        256B-descriptor cost), bf16 pairwise tree-reduce on DVE, scale by
        1/sqrt(nnz), PE-transpose, one matmul per chunk for c^T.
      phase 2: 50 ancestral DDPM steps over FOUR independent 128-column
        chains (one per chunk) in fp16 to hide the per-step serial
        latency: ph = w1s^T @ [x~; c] (2 matmuls into one PSUM tile), one
        silu [128,256] on ACT, eps-psum via 4 matmuls (A_t x~,
        sigma-folded noise, W2^T h halves), x-update on DVE:
        x~' = (pe + temb'/(-C_t)) * (-C_t).
  * Host: un-permute rows, emit [4096, 64].
"""
import sys

import numpy as np

if "/opt/trn_rl_repo" not in sys.path:
    sys.path.insert(0, "/opt/trn_rl_repo")

import ml_dtypes
import concourse.bass as bass
import concourse.mybir as mybir
import concourse.tile as tile
from concourse.tile_rust import add_dep_helper
from concourse import bacc
from concourse import ap_utils
from concourse.bass import MemorySpace, round_up_to_multiple
from concourse.bass_utils import run_bass_kernel_spmd
from concourse.masks import make_identity

F32 = mybir.dt.float32
F32R = mybir.dt.float32r
F16 = mybir.dt.float16
BF16 = mybir.dt.bfloat16
FP8 = mybir.dt.float8e4
I16 = mybir.dt.int16

T_STEPS = 50
D = 64
B = 4096
L = 200
V = 100000
NCORES = 8
BL = B // NCORES          # 512 rows per core
NCHUNK = BL // 128        # 4 chunks of 128 rows = 4 scan chains
NSEG = 4
SEG = 25000               # index range per segment
SEGR = SEG + 1            # +1 zero row


def _schedule_consts():
    steps = T_STEPS
    scale = 1000.0 / steps
    betas = np.linspace(scale * 1e-4, scale * 2e-2, steps, dtype=np.float64)
    alphas = 1.0 - betas
    acp = np.cumprod(alphas)
    acp_prev = np.append(1.0, acp[:-1])
    sqrt_recip = np.sqrt(1.0 / acp)
    sqrt_recipm1 = np.sqrt(1.0 / acp - 1.0)
    post_var = betas * (1.0 - acp_prev) / (1.0 - acp)
    post_logvar = np.log(np.append(post_var[1], post_var[1:]))
    coef1 = betas * np.sqrt(acp_prev) / (1.0 - acp)
    coef2 = (1.0 - acp_prev) * np.sqrt(alphas) / (1.0 - acp)
    A = coef1 * sqrt_recip + coef2
    C = coef1 * sqrt_recipm1
    S = np.exp(0.5 * post_logvar)
    S[0] = 0.0
    return A, C, S


def _timestep_emb(Wt, bt):
    half = D // 2
    freqs = np.exp(-np.log(10000.0) * np.arange(half, dtype=np.float64) / half)
    t = np.arange(T_STEPS, dtype=np.float64)
    args = t[:, None] * freqs[None, :]
    temb = np.concatenate([np.cos(args), np.sin(args)], axis=-1)
    return temb.astype(np.float32) @ Wt + bt  # [50, 64] (temb_t = row t)


def host_prep(inputs):
    seq = np.asarray(inputs["seq"]).astype(np.int64)
    item_emb = np.asarray(inputs["item_emb"], dtype=np.float32)
    W_enc = np.asarray(inputs["W_enc"], dtype=np.float32)
    b_enc = np.asarray(inputs["b_enc"], dtype=np.float32)
    Wt = np.asarray(inputs["Wt"], dtype=np.float32)
    bt = np.asarray(inputs["bt"], dtype=np.float32)
    Wc = np.asarray(inputs["Wc"], dtype=np.float32)
    bc = np.asarray(inputs["bc"], dtype=np.float32)
    W1 = np.asarray(inputs["W1"], dtype=np.float32)
    b1 = np.asarray(inputs["b1"], dtype=np.float32)
    W2 = np.asarray(inputs["W2"], dtype=np.float32)
    b2 = np.asarray(inputs["b2"], dtype=np.float32)
    init_noise = np.asarray(inputs["init_noise"], dtype=np.float32)
    step_noise = np.asarray(inputs["step_noise"], dtype=np.float32)

    assert np.abs(b1).max() == 0.0, "b1 must be zero (silu bias is folded out)"

    A, C, S = _schedule_consts()
    temb = _timestep_emb(Wt, bt).astype(np.float64)  # [50, 64]

    # ---- row packing: greedy bands minimizing per-band per-range max counts;
    # fattest band FIRST so the last chunk (shortest gathers) gates the scan.
    bucket = seq // SEG
    counts = np.stack([(bucket == k).sum(1) for k in range(NSEG)], 1)
    mx = counts.max(1)
    idx_desc = np.argsort(-mx, kind="stable")
    bands = [[] for _ in range(NCHUNK)]
    bmax = np.zeros((NCHUNK, NSEG), np.int64)
    for r in idx_desc:
        best, bestcost = None, None
        for b in range(NCHUNK):
            if len(bands[b]) >= NCORES * 128:
                continue
            cost = np.maximum(bmax[b], counts[r]).sum() - bmax[b].sum()
            if bestcost is None or cost < bestcost:
                best, bestcost = b, cost
        bands[best].append(r)
        bmax[best] = np.maximum(bmax[best], counts[r])
    border = np.argsort(-bmax.sum(1), kind="stable")   # fattest first
    order = np.concatenate([np.array(bands[b]) for b in border])
    rows = order.reshape(NCHUNK, NCORES, 128)          # [chunk, core, row]

    # fp8 table, 256B row pitch (cols 64:256 zero), +1 zero row per segment
    tbl = np.zeros((NSEG * SEGR, 256), ml_dtypes.float8_e4m3fn)
    for k in range(NSEG):
        tbl[k * SEGR: k * SEGR + SEG, 0:D] = item_emb[k * SEG: (k + 1) * SEG]

    G = counts[order].reshape(NCHUNK, NCORES * 128, NSEG).max(1)
    G = np.maximum(G, 1).astype(np.int64)              # [chunk, 4]

    # int16 gather index tiles per (core, chunk, range)
    idx16 = [[[None] * NSEG for _ in range(NCHUNK)] for _ in range(NCORES)]
    for c in range(NCHUNK):
        for n in range(NCORES):
            rs = rows[c, n]
            sq = seq[rs]
            bk = bucket[rs]
            for k in range(NSEG):
                g = int(G[c, k])
                val = np.full((128, g), SEG, np.int16)
                for p in range(128):
                    e = sq[p][bk[p] == k] - k * SEG
                    val[p, : len(e)] = e.astype(np.int16)
                # slot i = gg*128 + p  ->  idx tile [i%16, i//16]
                v = val.reshape(8, 16, g)              # [p//16, p%16, g]
                arr = np.transpose(v, (1, 2, 0)).reshape(16, g * 8)
                idx16[n][c][k] = np.ascontiguousarray(np.tile(arr, (8, 1)))

    wec = (W_enc[:, :D] @ Wc).astype(np.float32)
    bec = (b_enc[:D] @ Wc + bc).astype(np.float32).reshape(D, 1)
    # f16 const bundle [128, 384]: w1s | w2a | w2b
    cb16 = np.zeros((128, 384), np.float16)
    cb16[:, 0:256] = np.vstack([W1, W1])
    cb16[:, 256:320] = W2[0:128, :]
    cb16[:, 320:384] = W2[128:256, :]

    # per-step diagonal fold coefficients (built into diag blocks on-device)
    iaxc = np.zeros((D, T_STEPS), np.float32)
    iaxeff = np.empty(T_STEPS, np.float64)
    for t in range(T_STEPS):
        rat = np.float32(A[t] / (-C[t]))
        iaxc[:, t] = rat
        iaxeff[t] = np.float64(np.float16(rat))   # f16 diag as built
    Aeff = iaxeff * (-C)   # effective x passthrough after f16 rounding

    # noise+temb fold, feature-major per step i (t = 49-i):
    # x~' = (-C_t)*pe + nzf_i with
    # nzf_i = -Aeff_t*temb_t - C_t*b2 + S_t*n_i^T + temb_{t-1} (0 at t=0)
    per_core = []
    for n in range(NCORES):
        rws = rows[:, n, :].reshape(-1)
        nT = np.empty((T_STEPS, D, BL), np.float64)
        for i in range(T_STEPS):
            t = T_STEPS - 1 - i
            base = -Aeff[t] * temb[t] - C[t] * b2.astype(np.float64)
            if t > 0:
                base = base + temb[t - 1]
            nT[i] = base[:, None] + S[t] * step_noise[i][rws].T.astype(np.float64)
        noiseT = np.ascontiguousarray(
            nT.transpose(1, 0, 2).reshape(D, T_STEPS * BL)).astype(np.float16)
        x0T = np.ascontiguousarray(
            (init_noise[rws] + temb[T_STEPS - 1][None, :]).T).astype(np.float16)
        nnz = np.count_nonzero(seq[rws], axis=1).astype(np.float64)
        rsq = (1.0 / np.sqrt(np.maximum(nnz, 1.0))).astype(np.float32)
        rsqt = np.ascontiguousarray(rsq.reshape(NCHUNK, 128).T)   # [128, NCHUNK]
        # f32 const bundle [128, 119]: wec | bec | rsq | iaxc
        cb32 = np.zeros((128, 119), np.float32)
        cb32[0:D, 0:64] = wec
        cb32[0:D, 64:65] = bec
        cb32[:, 65:69] = rsqt
        cb32[0:D, 69:119] = iaxc
        # merged const bundle, f16-typed: [cb16 | cb32 viewed as f16]
        cb = np.concatenate([cb16, cb32.view(np.float16)], axis=1)
        core = dict(tbl=tbl, noiseT=noiseT, x0T=x0T, cb=np.ascontiguousarray(cb))
        for c in range(NCHUNK):
            # concat in gather order (fattest segment first)
            ks = sorted(range(NSEG), key=lambda k: -G[c, k])
            parts = [idx16[n][c][k] for k in ks]
            core[f"idxc_{c}"] = np.ascontiguousarray(np.concatenate(parts, 1))
        per_core.append((core, rws))

    consts = dict(A=A.astype(np.float32), C=C.astype(np.float32))
    return per_core, G, consts


def dma_gather_small(gp, out_ap, in_ap, idxs_ap, num_idxs, num_idxs_reg,
                     elem_size, elem_step, single_packet=False, queue_num=0):
    """nc.gpsimd.dma_gather without the elem_size_bytes%256 assert
    (transpose=False, DRAM source). elem_step*dtype must be %256."""
    assert idxs_ap.dtype == mybir.dt.int16
    assert in_ap.space == MemorySpace.DRAM
    assert idxs_ap.space == MemorySpace.SBUF
    assert out_ap.space == MemorySpace.SBUF
    assert ap_utils.ap_is_contiguous(out_ap.ap[1:])
    assert ap_utils.ap_is_contiguous(idxs_ap.ap[1:])
    assert in_ap.ap[-1][1] == out_ap.ap[-1][1] == elem_size
    assert out_ap.ap[0][1] * out_ap.ap[1][1] == round_up_to_multiple(num_idxs, 128)
    assert in_ap.ap[0][0] == elem_step
    stride_bytes = elem_step * mybir.dt.size(in_ap.dtype)
    assert stride_bytes % 256 == 0 and stride_bytes // 256 < 256
    _in_ap = gp.lower_ap_dma(in_ap, for_custom_bir_dma=True)
    _idxs_ap = gp.lower_ap(idxs_ap)
    _out_ap = gp.lower_ap(out_ap)
    return gp.add_instruction(
        mybir.InstDMAGatherAnt(
            name=gp.bass.get_next_instruction_name(),
            ins=[*_in_ap, _idxs_ap, gp.lower_val_access(gp.to_reg(num_idxs_reg))],
            outs=[_out_ap],
            transpose=False,
            num_idxs=num_idxs,
            elem_size=elem_size,
            stride_bytes_256=stride_bytes // 256,
            gen_mode=0,
            single_packet=single_packet,
            queue_num=queue_num,
            sbuf_tokens_per_rank=0,
            sbuf_free_dim_per_rank=0,
            sbuf_free_dim_pad_per_rank=0,
            sbuf_byte_offset=0,
        )
    )


def build_program(G, consts, N_WARM=55, NZ_PIECES=5):
    A, C = consts["A"], consts["C"]
    nc = bacc.Bacc("TRN2", target_bir_lowering=False, debug=False,
                   num_devices=NCORES)

    din = lambda name, shape, dt=F32: nc.dram_tensor(
        name, shape, dt, kind="ExternalInput").ap()
    tbl_d = din("tbl", [NSEG * SEGR, 256], FP8)
    noiseT_d = din("noiseT", [D, T_STEPS * BL], F16)
    x0T_d = din("x0T", [D, BL], F16)
    cb_d = din("cb", [128, 384 + 238], F16)
    idx_d = {}
    for c in range(NCHUNK):
        idx_d[c] = din(f"idxc_{c}", [128, 8 * int(G[c].sum())], I16)
    outT_d = nc.dram_tensor("outT", [D, BL], F16, kind="ExternalOutput").ap()

    Gmax = int(G.max())

    with tile.TileContext(nc) as tc:
        with (
            tc.tile_pool(name="const", bufs=1) as constp,
            tc.tile_pool(name="gidx", bufs=1) as gidxp,
            tc.tile_pool(name="gdst", bufs=5) as gdstp,
            tc.tile_pool(name="redb", bufs=4) as redb,
            tc.tile_pool(name="redp", bufs=6) as redp,
            tc.tile_pool(name="xcp", bufs=1) as xcp,
            tc.tile_pool(name="hp", bufs=6) as hp,
            tc.tile_pool(name="ps_t", bufs=1, space="PSUM") as ps_t,
            tc.tile_pool(name="ps_h", bufs=3, space="PSUM") as ps_h,
            tc.tile_pool(name="ps_e", bufs=4, space="PSUM") as ps_e,
        ):
            # ---- bundled consts (tile now, DMA issued after the idx loads)
            cbt = constp.tile([128, 384 + 238], F16, name="cbt")
            ident = constp.tile([128, 128], F32, name="ident")
            make_identity(nc, ident[:])
            w1s = cbt[:, 0:256]
            w2a = cbt[:, 256:320]
            w2b = cbt[:, 320:384]
            cb32 = cbt[:, 384:622].bitcast(F32)
            wec = cb32[0:D, 0:64]
            bec = cb32[0:D, 64:65]
            rsq = cb32[:, 65:69]
            iaxc = cb32[0:D, 69:119]

            # on-device diag blocks: iax (f16), per-chunk rsq diag (f32)
            # (tiles allocated here; ops emitted after the cbt DMA below)
            iax = constp.tile([D, T_STEPS * D], F16, name="iax")
            rsqd = [constp.tile([128, 128], F32, name=f"rsqd{c}")
                    for c in range(NCHUNK)]

            diag_jobs = []

            def build_diags():
                for c in range(NCHUNK):
                    diag_jobs.append(lambda c=c: nc.vector.tensor_scalar(
                        out=rsqd[c][:], in0=ident[:], scalar1=rsq[:, c:c + 1],
                        scalar2=None, op0=mybir.AluOpType.mult))
                for t in range(T_STEPS):
                    # on ACT (idle in the gather window; DVE is reduce-bound)
                    diag_jobs.append(lambda t=t: nc.scalar.activation(
                        iax[:, t * D:(t + 1) * D], ident[0:D, 0:D],
                        mybir.ActivationFunctionType.Identity,
                        scale=iaxc[:, t:t + 1]))

            def emit_diags(n):
                while n > 0 and diag_jobs:
                    diag_jobs.pop(0)()
                    n -= 1

            nz = constp.tile([D, T_STEPS * BL], F16, name="nz")
            xout = constp.tile([D, BL], F16, name="xout")
            xcq = [xcp.tile([128, 128], F16, name=f"xc{q}", tag=f"xc{q}")
                   for q in range(NCHUNK)]
            poolT = [constp.tile([D, 128], F32, name=f"poolT{q}")
                     for q in range(NCHUNK)]

            idx_t = {}
            # per-chunk idx col offset for segment k (gather order = G desc)
            idx_off = {}
            for c in range(NCHUNK):
                ks = sorted(range(NSEG), key=lambda k: -G[c, k])
                off = 0
                for k in ks:
                    idx_off[(c, k)] = off
                    off += 8 * int(G[c, k])

            def load_idx(c, split_first=0):
                gs = int(G[c].sum())
                if split_first:
                    # first gather's idx in its own tile, loaded first
                    s = 8 * split_first
                    ita = gidxp.tile([128, s], I16, name=f"it{c}a", tag=f"it{c}a")
                    nc.sync.dma_start(ita[:], idx_d[c][:, 0:s])
                    it = gidxp.tile([128, 8 * gs - s], I16, name=f"it{c}",
                                    tag=f"it{c}")
                    nc.sync.dma_start(it[:], idx_d[c][:, s:])
                    idx_t[c] = (ita, it, s)
                else:
                    it = gidxp.tile([128, 8 * gs], I16, name=f"it{c}",
                                    tag=f"it{c}")
                    nc.sync.dma_start(it[:], idx_d[c][:])
                    idx_t[c] = (None, it, 0)

            def idx_ap(c, off, width):
                ita, it, s = idx_t[c]
                if ita is not None and off < s:
                    assert off + width <= s
                    return ita[:, off:off + width]
                return it[:, off - s:off - s + width]

            def do_gather(c, k, soff, g):
                off = idx_off[(c, k)] + 8 * soff
                dst = gdstp.tile([128, Gmax * D], FP8, name="dst", tag="dst")
                return dst, dma_gather_small(
                    nc.gpsimd,
                    dst[:, : g * D].rearrange("p (g d) -> p g d", g=g, d=D),
                    tbl_d[k * SEGR:(k + 1) * SEGR, 0:D],
                    idx_ap(c, off, 8 * g), 128 * g, 128 * g, D, 256)

            def do_reduce(g, dst, acc):
                """fp8 pair-add into bf16, bf16 tree to 2, mixed-add to f32."""
                ops = []
                m = g // 2
                if m == 0:
                    sk = redp.tile([128, D], F32, name="sk", tag="rk")
                    ops.append(nc.vector.tensor_copy(sk[:], dst[:, 0:D]))
                else:
                    red = redb.tile([128, (Gmax // 2 + 1) * D], BF16,
                                    name="red", tag="red")
                    op = nc.vector.tensor_tensor(
                        out=red[:, : m * D], in0=dst[:, : m * D],
                        in1=dst[:, m * D: 2 * m * D], op=mybir.AluOpType.add)
                    ops.append(op)
                    w = m
                    if g % 2:
                        ops.append(nc.vector.tensor_copy(
                            red[:, m * D:(m + 1) * D], dst[:, (g - 1) * D:g * D]))
                        w = m + 1
                    while w > 2:
                        mm2 = w // 2
                        ops.append(nc.vector.tensor_tensor(
                            out=red[:, : mm2 * D], in0=red[:, : mm2 * D],
                            in1=red[:, (w - mm2) * D: w * D],
                            op=mybir.AluOpType.add))
                        w = w - mm2
                    sk = redp.tile([128, D], F32, name="sk", tag="rk")
                    if w == 2:
                        ops.append(nc.vector.tensor_tensor(
                            out=sk[:], in0=red[:, 0:D], in1=red[:, D:2 * D],
                            op=mybir.AluOpType.add))
                    else:
                        ops.append(nc.vector.tensor_copy(sk[:], red[:, 0:D]))
                if acc is None:
                    return sk, ops
                acc2 = redp.tile([128, D], F32, name="acc2", tag="rk")
                ops.append(nc.vector.tensor_tensor(
                    out=acc2[:], in0=acc[:], in1=sk[:], op=mybir.AluOpType.add))
                return acc2, ops

            def do_finish_chunk(c, acc):
                # transpose + rsq fold in one regular matmul:
                # pt = acc.T @ diag(rsq_c)
                pt = ps_t.tile([D, 128], F32, name="pt", tag="pt")
                nc.tensor.matmul(out=pt[:], lhsT=acc[:], rhs=rsqd[c][:],
                                 start=True, stop=True)
                nc.scalar.copy(poolT[c][:], pt[:])
                # conditioning for chain c
                pc = ps_t.tile([D, 128], F32, name="pc", tag="pt")
                nc.tensor.matmul(out=pc[:], lhsT=wec, rhs=poolT[c][:],
                                 start=True, stop=True)
                nc.scalar.activation(xcq[c][D:128, :], pc[:],
                                     mybir.ActivationFunctionType.Identity,
                                     bias=bec)

            # ---- phase 1: gathers + reduces, pipelined; within each chunk
            # the fattest segment first (leanest last => shortest tail).
            # The very first gather is split in two so its descriptor-gen
            # overlaps its own transfer.
            jobs = []
            for c in range(NCHUNK):
                ks = sorted(range(NSEG), key=lambda k: -G[c, k])
                jobs += [(c, k, 0, int(G[c, k])) for k in ks]
            # split the last job so the final reduce tail is shorter
            cl, kl, _, gl = jobs[-1]
            jobs[-1:] = [(cl, kl, 0, gl // 2), (cl, kl, gl // 2, gl - gl // 2)]
            left = {c: sum(1 for jb in jobs if jb[0] == c) for c in range(NCHUNK)}
            PIPE_G = 4
            load_idx(0, split_first=int(G[0, jobs[0][1]]))
            load_idx(1)
            nc.sync.dma_start(cbt[:], cb_d[:])
            build_diags()
            for q in range(NCHUNK):
                nc.sync.dma_start(xcq[q][0:D, :], x0T_d[:, q * 128:(q + 1) * 128])
            gdsts = {}
            for j in range(PIPE_G):
                gdsts[j] = do_gather(*jobs[j])
            accs = {c: None for c in range(NCHUNK)}
            warm_dep = None
            last_gather = None
            loaded = 2
            for j in range(len(jobs)):
                c, k, soff, g = jobs[j]
                if j + PIPE_G < len(jobs):
                    cn = jobs[j + PIPE_G][0]
                    if cn >= loaded:
                        load_idx(cn)
                        loaded = cn + 1
                    gdsts[j + PIPE_G] = do_gather(*jobs[j + PIPE_G])
                dst, ginst = gdsts.pop(j)
                if j == len(jobs) - 1:
                    last_gather = ginst
                accs[c], ops = do_reduce(g, dst, accs[c])
                emit_diags(4)
                if j == len(jobs) - 2:
                    warm_dep = ops[0]
                left[c] -= 1
                if left[c] == 0:
                    if j == len(jobs) - 1:
                        # PE warm-up BEFORE the last chunk-finish so it runs
                        # during the final gather/reduce, not after pc3 (the
                        # in-order PE queue would put it on the critical path)
                        warm_t = ps_t.tile([D, 128], F32, name="warm_t",
                                           tag="pt")
                        for i in range(N_WARM):
                            wm = nc.tensor.matmul(out=warm_t[:], lhsT=w2a,
                                                  rhs=w1s[:, 0:128],
                                                  start=True, stop=True)
                            if i == 0 and warm_dep is not None:
                                add_dep_helper(wm.ins, warm_dep.ins, sync=False,
                                               reason="warm near last reduce")
                    do_finish_chunk(c, accs[c])

            # ---- deferred DMAs (gated behind the last gather)
            def gated_dma(dst_ap, src_ap):
                inst = nc.sync.dma_start(dst_ap, src_ap)
                add_dep_helper(inst.ins, last_gather.ins, sync=True,
                               reason="defer until gathers done")
                return inst

            npc = T_STEPS // NZ_PIECES
            for p in range(NZ_PIECES):
                gated_dma(nz[:, p * npc * BL:(p + 1) * npc * BL],
                          noiseT_d[:, p * npc * BL:(p + 1) * npc * BL])

            # ---- phase 2: 50 steps, four 128-col chains in lockstep.
            # Matmuls grouped by stationary weight (5 LdWeights per wave);
            # noise+temb folded into the DVE x-update.
            for k in range(T_STEPS):
                live = [(q, k) for q in range(NCHUNK)]
                phs = {}
                for q, i in live:
                    phs[q] = ps_h.tile([128, 256], F32, name=f"ph{q}",
                                       tag="ph")
                for q, i in live:
                    nc.tensor.matmul(out=phs[q][:, 0:128], lhsT=w1s[:, 0:128],
                                     rhs=xcq[q][:], start=True, stop=True)
                for q, i in live:
                    nc.tensor.matmul(out=phs[q][:, 128:256],
                                     lhsT=w1s[:, 128:256],
                                     rhs=xcq[q][:], start=True, stop=True)
                hts = {}
                for q, i in live:
                    ht = hp.tile([128, 256], F16, name=f"h{q}", tag="h")
                    nc.scalar.activation(ht[:], phs[q][:],
                                         mybir.ActivationFunctionType.Silu)
                    hts[q] = ht
                pes = {}
                for q, i in live:
                    pes[q] = ps_e.tile([D, 128], F32, name=f"pe{q}", tag="pe")
                for q, i in live:
                    t = T_STEPS - 1 - i
                    nc.tensor.matmul(out=pes[q][:],
                                     lhsT=iax[:, t * D:(t + 1) * D],
                                     rhs=xcq[q][0:D, :], start=True, stop=False)
                for q, i in live:
                    nc.tensor.matmul(out=pes[q][:], lhsT=w2a,
                                     rhs=hts[q][:, 0:128],
                                     start=False, stop=False)
                for q, i in live:
                    nc.tensor.matmul(out=pes[q][:], lhsT=w2b,
                                     rhs=hts[q][:, 128:256],
                                     start=False, stop=True)
                for q, i in live:
                    t = T_STEPS - 1 - i
                    col = i * BL + q * 128
                    dst = (xcq[q][0:D, :] if i < T_STEPS - 1
                           else xout[:, q * 128:(q + 1) * 128])
                    nc.vector.scalar_tensor_tensor(
                        out=dst, in0=pes[q][:],
                        scalar=-float(C[t]), in1=nz[:, col:col + 128],
                        op0=mybir.AluOpType.mult, op1=mybir.AluOpType.add)

            nc.sync.dma_start(outT_d[:], xout[:])

    nc.compile()
    return nc


_CACHE = {}


def _get_program(G, consts):
    key = tuple(G.reshape(-1).tolist())
    if key not in _CACHE:
        _CACHE[key] = build_program(G, consts)
    return _CACHE[key]


def kernel(**inputs):
    per_core, G, consts = host_prep(inputs)
    nc = _get_program(G, consts)
    in_maps = [core for core, _ in per_core]
    res = run_bass_kernel_spmd(nc, in_maps, list(range(NCORES)))
    out = np.zeros((B, D), np.float32)
    for n in range(NCORES):
        _, rws = per_core[n]
        out[rws] = np.asarray(res.results[n]["outT"]).astype(np.float32).T
    return out



# revision 54
# speedup vs baseline: 1.0016x; 1.0016x over previous
"""Trainium2 Bass kernel for nn_DDPMVAEQueryEncoder.

Strategy (data-parallel over batch, 8 cores):
  * Host: bucket/pack rows into 4 bands of 1024 (fattest band first) to
    minimize gather padding; build int16 gather-index tiles; fold all
    weight-only matmuls; fold timestep embeddings into the x-state
    (x~ = x + temb_t) with per-step corrections folded into the noise
    tensor; precompute 1/sqrt(nnz) per row.
  * Device per core (512 batch rows):
      phase 1: bf16 embedding table with 256B row pitch gathered via
        128B-element dma_gather (one descriptor per lookup at half the
        256B-descriptor cost), bf16 pairwise tree-reduce on DVE, scale by
        1/sqrt(nnz), PE-transpose, one matmul per chunk for c^T.
      phase 2: 50 ancestral DDPM steps over FOUR independent 128-column
        chains (one per chunk) in fp16 to hide the per-step serial
        latency: ph = w1s^T @ [x~; c] (2 matmuls into one PSUM tile), one
        silu [128,256] on ACT, eps-psum via 4 matmuls (A_t x~,
        sigma-folded noise, W2^T h halves), x-update on DVE:
        x~' = (pe + temb'/(-C_t)) * (-C_t).
  * Host: un-permute rows, emit [4096, 64].
"""
import sys

import numpy as np

if "/opt/trn_rl_repo" not in sys.path:
    sys.path.insert(0, "/opt/trn_rl_repo")

import ml_dtypes
import concourse.bass as bass
import concourse.mybir as mybir
import concourse.tile as tile
from concourse.tile_rust import add_dep_helper
from concourse import bacc
from concourse import ap_utils
from concourse.bass import MemorySpace, round_up_to_multiple
from concourse.bass_utils import run_bass_kernel_spmd
from concourse.masks import make_identity

F32 = mybir.dt.float32
F32R = mybir.dt.float32r
F16 = mybir.dt.float16
BF16 = mybir.dt.bfloat16
FP8 = mybir.dt.float8e4
I16 = mybir.dt.int16

T_STEPS = 50
D = 64
B = 4096
L = 200
V = 100000
NCORES = 8
BL = B // NCORES          # 512 rows per core
NCHUNK = BL // 128        # 4 chunks of 128 rows = 4 scan chains
NSEG = 4
SEG = 25000               # index range per segment
SEGR = SEG + 1            # +1 zero row


def _schedule_consts():
    steps = T_STEPS
    scale = 1000.0 / steps
    betas = np.linspace(scale * 1e-4, scale * 2e-2, steps, dtype=np.float64)
    alphas = 1.0 - betas
    acp = np.cumprod(alphas)
    acp_prev = np.append(1.0, acp[:-1])
    sqrt_recip = np.sqrt(1.0 / acp)
    sqrt_recipm1 = np.sqrt(1.0 / acp - 1.0)
    post_var = betas * (1.0 - acp_prev) / (1.0 - acp)
    post_logvar = np.log(np.append(post_var[1], post_var[1:]))
    coef1 = betas * np.sqrt(acp_prev) / (1.0 - acp)
    coef2 = (1.0 - acp_prev) * np.sqrt(alphas) / (1.0 - acp)
    A = coef1 * sqrt_recip + coef2
    C = coef1 * sqrt_recipm1
    S = np.exp(0.5 * post_logvar)
    S[0] = 0.0
    return A, C, S


def _timestep_emb(Wt, bt):
    half = D // 2
    freqs = np.exp(-np.log(10000.0) * np.arange(half, dtype=np.float64) / half)
    t = np.arange(T_STEPS, dtype=np.float64)
    args = t[:, None] * freqs[None, :]
    temb = np.concatenate([np.cos(args), np.sin(args)], axis=-1)
    return temb.astype(np.float32) @ Wt + bt  # [50, 64] (temb_t = row t)


def host_prep(inputs):
    seq = np.asarray(inputs["seq"]).astype(np.int64)
    item_emb = np.asarray(inputs["item_emb"], dtype=np.float32)
    W_enc = np.asarray(inputs["W_enc"], dtype=np.float32)
    b_enc = np.asarray(inputs["b_enc"], dtype=np.float32)
    Wt = np.asarray(inputs["Wt"], dtype=np.float32)
    bt = np.asarray(inputs["bt"], dtype=np.float32)
    Wc = np.asarray(inputs["Wc"], dtype=np.float32)
    bc = np.asarray(inputs["bc"], dtype=np.float32)
    W1 = np.asarray(inputs["W1"], dtype=np.float32)
    b1 = np.asarray(inputs["b1"], dtype=np.float32)
    W2 = np.asarray(inputs["W2"], dtype=np.float32)
    b2 = np.asarray(inputs["b2"], dtype=np.float32)
    init_noise = np.asarray(inputs["init_noise"], dtype=np.float32)
    step_noise = np.asarray(inputs["step_noise"], dtype=np.float32)

    assert np.abs(b1).max() == 0.0, "b1 must be zero (silu bias is folded out)"

    A, C, S = _schedule_consts()
    temb = _timestep_emb(Wt, bt).astype(np.float64)  # [50, 64]

    # ---- row packing: greedy bands minimizing per-band per-range max counts;
    # fattest band FIRST so the last chunk (shortest gathers) gates the scan.
    bucket = seq // SEG
    counts = np.stack([(bucket == k).sum(1) for k in range(NSEG)], 1)
    mx = counts.max(1)
    idx_desc = np.argsort(-mx, kind="stable")
    bands = [[] for _ in range(NCHUNK)]
    bmax = np.zeros((NCHUNK, NSEG), np.int64)
    for r in idx_desc:
        best, bestcost = None, None
        for b in range(NCHUNK):
            if len(bands[b]) >= NCORES * 128:
                continue
            cost = np.maximum(bmax[b], counts[r]).sum() - bmax[b].sum()
            if bestcost is None or cost < bestcost:
                best, bestcost = b, cost
        bands[best].append(r)
        bmax[best] = np.maximum(bmax[best], counts[r])
    border = np.argsort(-bmax.sum(1), kind="stable")   # fattest first
    order = np.concatenate([np.array(bands[b]) for b in border])
    rows = order.reshape(NCHUNK, NCORES, 128)          # [chunk, core, row]

    # fp8 table, 256B row pitch (cols 64:256 zero), +1 zero row per segment
    tbl = np.zeros((NSEG * SEGR, 256), ml_dtypes.float8_e4m3fn)
    for k in range(NSEG):
        tbl[k * SEGR: k * SEGR + SEG, 0:D] = item_emb[k * SEG: (k + 1) * SEG]

    G = counts[order].reshape(NCHUNK, NCORES * 128, NSEG).max(1)
    G = np.maximum(G, 1).astype(np.int64)              # [chunk, 4]

    # int16 gather index tiles per (core, chunk, range)
    idx16 = [[[None] * NSEG for _ in range(NCHUNK)] for _ in range(NCORES)]
    for c in range(NCHUNK):
        for n in range(NCORES):
            rs = rows[c, n]
            sq = seq[rs]
            bk = bucket[rs]
            for k in range(NSEG):
                g = int(G[c, k])
                val = np.full((128, g), SEG, np.int16)
                for p in range(128):
                    e = sq[p][bk[p] == k] - k * SEG
                    val[p, : len(e)] = e.astype(np.int16)
                # slot i = gg*128 + p  ->  idx tile [i%16, i//16]
                v = val.reshape(8, 16, g)              # [p//16, p%16, g]
                arr = np.transpose(v, (1, 2, 0)).reshape(16, g * 8)
                idx16[n][c][k] = np.ascontiguousarray(np.tile(arr, (8, 1)))

    wec = (W_enc[:, :D] @ Wc).astype(np.float32)
    bec = (b_enc[:D] @ Wc + bc).astype(np.float32).reshape(D, 1)
    # f16 const bundle [128, 384]: w1s | w2a | w2b
    cb16 = np.zeros((128, 384), np.float16)
    cb16[:, 0:256] = np.vstack([W1, W1])
    cb16[:, 256:320] = W2[0:128, :]
    cb16[:, 320:384] = W2[128:256, :]

    # per-step diagonal fold coefficients (built into diag blocks on-device)
    iaxc = np.zeros((D, T_STEPS), np.float32)
    iaxeff = np.empty(T_STEPS, np.float64)
    for t in range(T_STEPS):
        rat = np.float32(A[t] / (-C[t]))
        iaxc[:, t] = rat
        iaxeff[t] = np.float64(np.float16(rat))   # f16 diag as built
    Aeff = iaxeff * (-C)   # effective x passthrough after f16 rounding

    # noise+temb fold, feature-major per step i (t = 49-i):
    # x~' = (-C_t)*pe + nzf_i with
    # nzf_i = -Aeff_t*temb_t - C_t*b2 + S_t*n_i^T + temb_{t-1} (0 at t=0)
    per_core = []
    for n in range(NCORES):
        rws = rows[:, n, :].reshape(-1)
        nT = np.empty((T_STEPS, D, BL), np.float64)
        for i in range(T_STEPS):
            t = T_STEPS - 1 - i
            base = -Aeff[t] * temb[t] - C[t] * b2.astype(np.float64)
            if t > 0:
                base = base + temb[t - 1]
            nT[i] = base[:, None] + S[t] * step_noise[i][rws].T.astype(np.float64)
        noiseT = np.ascontiguousarray(
            nT.transpose(1, 0, 2).reshape(D, T_STEPS * BL)).astype(np.float16)
        x0T = np.ascontiguousarray(
            (init_noise[rws] + temb[T_STEPS - 1][None, :]).T).astype(np.float16)
        nnz = np.count_nonzero(seq[rws], axis=1).astype(np.float64)
        rsq = (1.0 / np.sqrt(np.maximum(nnz, 1.0))).astype(np.float32)
        rsqt = np.ascontiguousarray(rsq.reshape(NCHUNK, 128).T)   # [128, NCHUNK]
        # f32 const bundle [128, 119]: wec | bec | rsq | iaxc
        cb32 = np.zeros((128, 119), np.float32)
        cb32[0:D, 0:64] = wec
        cb32[0:D, 64:65] = bec
        cb32[:, 65:69] = rsqt
        cb32[0:D, 69:119] = iaxc
        # merged const bundle, f16-typed: [cb16 | cb32 viewed as f16]
        cb = np.concatenate([cb16, cb32.view(np.float16)], axis=1)
        core = dict(tbl=tbl, noiseT=noiseT, x0T=x0T, cb=np.ascontiguousarray(cb))
        for c in range(NCHUNK):
            # concat in gather order (fattest segment first)
            ks = sorted(range(NSEG), key=lambda k: -G[c, k])
            parts = [idx16[n][c][k] for k in ks]
            core[f"idxc_{c}"] = np.ascontiguousarray(np.concatenate(parts, 1))
        per_core.append((core, rws))

    consts = dict(A=A.astype(np.float32), C=C.astype(np.float32))
    return per_core, G, consts


def dma_gather_small(gp, out_ap, in_ap, idxs_ap, num_idxs, num_idxs_reg,
                     elem_size, elem_step, single_packet=False, queue_num=0):
    """nc.gpsimd.dma_gather without the elem_size_bytes%256 assert
    (transpose=False, DRAM source). elem_step*dtype must be %256."""
    assert idxs_ap.dtype == mybir.dt.int16
    assert in_ap.space == MemorySpace.DRAM
    assert idxs_ap.space == MemorySpace.SBUF
    assert out_ap.space == MemorySpace.SBUF
    assert ap_utils.ap_is_contiguous(out_ap.ap[1:])
    assert ap_utils.ap_is_contiguous(idxs_ap.ap[1:])
    assert in_ap.ap[-1][1] == out_ap.ap[-1][1] == elem_size
    assert out_ap.ap[0][1] * out_ap.ap[1][1] == round_up_to_multiple(num_idxs, 128)
    assert in_ap.ap[0][0] == elem_step
    stride_bytes = elem_step * mybir.dt.size(in_ap.dtype)
    assert stride_bytes % 256 == 0 and stride_bytes // 256 < 256
    _in_ap = gp.lower_ap_dma(in_ap, for_custom_bir_dma=True)
    _idxs_ap = gp.lower_ap(idxs_ap)
    _out_ap = gp.lower_ap(out_ap)
    return gp.add_instruction(
        mybir.InstDMAGatherAnt(
            name=gp.bass.get_next_instruction_name(),
            ins=[*_in_ap, _idxs_ap, gp.lower_val_access(gp.to_reg(num_idxs_reg))],
            outs=[_out_ap],
            transpose=False,
            num_idxs=num_idxs,
            elem_size=elem_size,
            stride_bytes_256=stride_bytes // 256,
            gen_mode=0,
            single_packet=single_packet,
            queue_num=queue_num,
            sbuf_tokens_per_rank=0,
            sbuf_free_dim_per_rank=0,
            sbuf_free_dim_pad_per_rank=0,
            sbuf_byte_offset=0,
        )
    )


def build_program(G, consts, N_WARM=55, NZ_PIECES=5):
    A, C = consts["A"], consts["C"]
    nc = bacc.Bacc("TRN2", target_bir_lowering=False, debug=False,
                   num_devices=NCORES)

    din = lambda name, shape, dt=F32: nc.dram_tensor(
        name, shape, dt, kind="ExternalInput").ap()
    tbl_d = din("tbl", [NSEG * SEGR, 256], FP8)
    noiseT_d = din("noiseT", [D, T_STEPS * BL], F16)
    x0T_d = din("x0T", [D, BL], F16)
    cb_d = din("cb", [128, 384 + 238], F16)
    idx_d = {}
    for c in range(NCHUNK):
        idx_d[c] = din(f"idxc_{c}", [128, 8 * int(G[c].sum())], I16)
    outT_d = nc.dram_tensor("outT", [D, BL], F16, kind="ExternalOutput").ap()

    Gmax = int(G.max())

    with tile.TileContext(nc) as tc:
        with (
            tc.tile_pool(name="const", bufs=1) as constp,
            tc.tile_pool(name="gidx", bufs=1) as gidxp,
            tc.tile_pool(name="gdst", bufs=5) as gdstp,
            tc.tile_pool(name="redb", bufs=4) as redb,
            tc.tile_pool(name="redp", bufs=6) as redp,
            tc.tile_pool(name="xcp", bufs=1) as xcp,
            tc.tile_pool(name="hp", bufs=6) as hp,
            tc.tile_pool(name="ps_t", bufs=1, space="PSUM") as ps_t,
            tc.tile_pool(name="ps_h", bufs=3, space="PSUM") as ps_h,
            tc.tile_pool(name="ps_e", bufs=4, space="PSUM") as ps_e,
        ):
            # ---- bundled consts (tile now, DMA issued after the idx loads)
            cbt = constp.tile([128, 384 + 238], F16, name="cbt")
            ident = constp.tile([128, 128], F32, name="ident")
            make_identity(nc, ident[:])
            w1s = cbt[:, 0:256]
            w2a = cbt[:, 256:320]
            w2b = cbt[:, 320:384]
            cb32 = cbt[:, 384:622].bitcast(F32)
            wec = cb32[0:D, 0:64]
            bec = cb32[0:D, 64:65]
            rsq = cb32[:, 65:69]
            iaxc = cb32[0:D, 69:119]

            # on-device diag blocks: iax (f16), per-chunk rsq diag (f32)
            # (tiles allocated here; ops emitted after the cbt DMA below)
            iax = constp.tile([D, T_STEPS * D], F16, name="iax")
            rsqd = [constp.tile([128, 128], F32, name=f"rsqd{c}")
                    for c in range(NCHUNK)]

            diag_jobs = []

            def build_diags():
                for c in range(NCHUNK):
                    diag_jobs.append(lambda c=c: nc.vector.tensor_scalar(
                        out=rsqd[c][:], in0=ident[:], scalar1=rsq[:, c:c + 1],
                        scalar2=None, op0=mybir.AluOpType.mult))
                for t in range(T_STEPS):
                    # on ACT (idle in the gather window; DVE is reduce-bound)
                    diag_jobs.append(lambda t=t: nc.scalar.activation(
                        iax[:, t * D:(t + 1) * D], ident[0:D, 0:D],
                        mybir.ActivationFunctionType.Identity,
                        scale=iaxc[:, t:t + 1]))

            def emit_diags(n):
                while n > 0 and diag_jobs:
                    diag_jobs.pop(0)()
                    n -= 1

            nz = constp.tile([D, T_STEPS * BL], F16, name="nz")
            xout = constp.tile([D, BL], F16, name="xout")
            xcq = [xcp.tile([128, 128], F16, name=f"xc{q}", tag=f"xc{q}")
                   for q in range(NCHUNK)]
            poolT = [constp.tile([D, 128], F32, name=f"poolT{q}")
                     for q in range(NCHUNK)]

            idx_t = {}
            # per-chunk idx col offset for segment k (gather order = G desc)
            idx_off = {}
            for c in range(NCHUNK):
                ks = sorted(range(NSEG), key=lambda k: -G[c, k])
                off = 0
                for k in ks:
                    idx_off[(c, k)] = off
                    off += 8 * int(G[c, k])

            def load_idx(c, split_first=0):
                gs = int(G[c].sum())
                if split_first:
                    # first gather's idx in its own tile, loaded first
                    s = 8 * split_first
                    ita = gidxp.tile([128, s], I16, name=f"it{c}a", tag=f"it{c}a")
                    nc.sync.dma_start(ita[:], idx_d[c][:, 0:s])
                    it = gidxp.tile([128, 8 * gs - s], I16, name=f"it{c}",
                                    tag=f"it{c}")
                    nc.sync.dma_start(it[:], idx_d[c][:, s:])
                    idx_t[c] = (ita, it, s)
                else:
                    it = gidxp.tile([128, 8 * gs], I16, name=f"it{c}",
                                    tag=f"it{c}")
                    nc.sync.dma_start(it[:], idx_d[c][:])
                    idx_t[c] = (None, it, 0)

            def idx_ap(c, off, width):
                ita, it, s = idx_t[c]
                if ita is not None and off < s:
                    assert off + width <= s
                    return ita[:, off:off + width]
                return it[:, off - s:off - s + width]

            def do_gather(c, k, soff, g):
                off = idx_off[(c, k)] + 8 * soff
                dst = gdstp.tile([128, Gmax * D], FP8, name="dst", tag="dst")
                return dst, dma_gather_small(
                    nc.gpsimd,
                    dst[:, : g * D].rearrange("p (g d) -> p g d", g=g, d=D),
                    tbl_d[k * SEGR:(k + 1) * SEGR, 0:D],
                    idx_ap(c, off, 8 * g), 128 * g, 128 * g, D, 256)

            def do_reduce(g, dst, acc):
                """fp8 pair-add into bf16, bf16 tree to 2, mixed-add to f32."""
                ops = []
                m = g // 2
                if m == 0:
                    sk = redp.tile([128, D], F32, name="sk", tag="rk")
                    ops.append(nc.vector.tensor_copy(sk[:], dst[:, 0:D]))
                else:
                    red = redb.tile([128, (Gmax // 2 + 1) * D], BF16,
                                    name="red", tag="red")
                    op = nc.vector.tensor_tensor(
                        out=red[:, : m * D], in0=dst[:, : m * D],
                        in1=dst[:, m * D: 2 * m * D], op=mybir.AluOpType.add)
                    ops.append(op)
                    w = m
                    if g % 2:
                        ops.append(nc.vector.tensor_copy(
                            red[:, m * D:(m + 1) * D], dst[:, (g - 1) * D:g * D]))
                        w = m + 1
                    while w > 2:
                        mm2 = w // 2
                        ops.append(nc.vector.tensor_tensor(
                            out=red[:, : mm2 * D], in0=red[:, : mm2 * D],
                            in1=red[:, (w - mm2) * D: w * D],
                            op=mybir.AluOpType.add))
                        w = w - mm2
                    sk = redp.tile([128, D], F32, name="sk", tag="rk")
                    if w == 2:
                        ops.append(nc.vector.tensor_tensor(
                            out=sk[:], in0=red[:, 0:D], in1=red[:, D:2 * D],
                            op=mybir.AluOpType.add))
                    else:
                        ops.append(nc.vector.tensor_copy(sk[:], red[:, 0:D]))
                if acc is None:
                    return sk, ops
                acc2 = redp.tile([128, D], F32, name="acc2", tag="rk")
                ops.append(nc.vector.tensor_tensor(
                    out=acc2[:], in0=acc[:], in1=sk[:], op=mybir.AluOpType.add))
                return acc2, ops

            def do_finish_chunk(c, acc):
                # transpose + rsq fold in one regular matmul:
                # pt = acc.T @ diag(rsq_c)
                pt = ps_t.tile([D, 128], F32, name="pt", tag="pt")
                nc.tensor.matmul(out=pt[:], lhsT=acc[:], rhs=rsqd[c][:],
                                 start=True, stop=True)
                nc.scalar.copy(poolT[c][:], pt[:])
                # conditioning for chain c
                pc = ps_t.tile([D, 128], F32, name="pc", tag="pt")
                nc.tensor.matmul(out=pc[:], lhsT=wec, rhs=poolT[c][:],
                                 start=True, stop=True)
                nc.scalar.activation(xcq[c][D:128, :], pc[:],
                                     mybir.ActivationFunctionType.Identity,
                                     bias=bec)

            # ---- phase 1: gathers + reduces, pipelined; within each chunk
            # the fattest segment first (leanest last => shortest tail).
            # The very first gather is split in two so its descriptor-gen
            # overlaps its own transfer.
            jobs = []
            for c in range(NCHUNK):
                ks = sorted(range(NSEG), key=lambda k: -G[c, k])
                jobs += [(c, k, 0, int(G[c, k])) for k in ks]
            # split the FIRST job: a small leading sub-gather primes the
            # DMA pipe earlier (short descgen before the first transfer)
            c0, k0, _, g0 = jobs[0]
            SPLIT0 = 12
            jobs[0:1] = [(c0, k0, 0, SPLIT0), (c0, k0, SPLIT0, g0 - SPLIT0)]
            # split the last job so the final reduce tail is shorter
            cl, kl, _, gl = jobs[-1]
            jobs[-1:] = [(cl, kl, 0, gl // 2), (cl, kl, gl // 2, gl - gl // 2)]
            left = {c: sum(1 for jb in jobs if jb[0] == c) for c in range(NCHUNK)}
            PIPE_G = 4
            load_idx(0, split_first=int(G[0, jobs[0][1]]))
            load_idx(1)
            nc.sync.dma_start(cbt[:], cb_d[:])
            build_diags()
            for q in range(NCHUNK):
                nc.sync.dma_start(xcq[q][0:D, :], x0T_d[:, q * 128:(q + 1) * 128])
            gdsts = {}
            for j in range(PIPE_G):
                gdsts[j] = do_gather(*jobs[j])
            accs = {c: None for c in range(NCHUNK)}
            warm_dep = None
            last_gather = None
            loaded = 2
            for j in range(len(jobs)):
                c, k, soff, g = jobs[j]
                if j + PIPE_G < len(jobs):
                    cn = jobs[j + PIPE_G][0]
                    if cn >= loaded:
                        load_idx(cn)
                        loaded = cn + 1
                    gdsts[j + PIPE_G] = do_gather(*jobs[j + PIPE_G])
                dst, ginst = gdsts.pop(j)
                if j == len(jobs) - 1:
                    last_gather = ginst
                accs[c], ops = do_reduce(g, dst, accs[c])
                emit_diags(4)
                if j == len(jobs) - 2:
                    warm_dep = ops[0]
                left[c] -= 1
                if left[c] == 0:
                    if j == len(jobs) - 1:
                        # PE warm-up BEFORE the last chunk-finish so it runs
                        # during the final gather/reduce, not after pc3 (the
                        # in-order PE queue would put it on the critical path)
                        warm_t = ps_t.tile([D, 128], F32, name="warm_t",
                                           tag="pt")
                        for i in range(N_WARM):
                            wm = nc.tensor.matmul(out=warm_t[:], lhsT=w2a,
                                                  rhs=w1s[:, 0:128],
                                                  start=True, stop=True)
                            if i == 0 and warm_dep is not None:
                                add_dep_helper(wm.ins, warm_dep.ins, sync=False,
                                               reason="warm near last reduce")
                    do_finish_chunk(c, accs[c])

            # ---- deferred DMAs (gated behind the last gather)
            def gated_dma(dst_ap, src_ap):
                inst = nc.sync.dma_start(dst_ap, src_ap)
                add_dep_helper(inst.ins, last_gather.ins, sync=True,
                               reason="defer until gathers done")
                return inst

            npc = T_STEPS // NZ_PIECES
            for p in range(NZ_PIECES):
                gated_dma(nz[:, p * npc * BL:(p + 1) * npc * BL],
                          noiseT_d[:, p * npc * BL:(p + 1) * npc * BL])

            # ---- phase 2: 50 steps, four 128-col chains in lockstep.
            # Matmuls grouped by stationary weight (5 LdWeights per wave);
            # noise+temb folded into the DVE x-update.
            for k in range(T_STEPS):
                live = [(q, k) for q in range(NCHUNK)]
                phs = {}
                for q, i in live:
                    phs[q] = ps_h.tile([128, 256], F32, name=f"ph{q}",
                                       tag="ph")
                for q, i in live:
                    nc.tensor.matmul(out=phs[q][:, 0:128], lhsT=w1s[:, 0:128],
                                     rhs=xcq[q][:], start=True, stop=True)
                for q, i in live:
                    nc.tensor.matmul(out=phs[q][:, 128:256],
                                     lhsT=w1s[:, 128:256],
                                     rhs=xcq[q][:], start=True, stop=True)
                hts = {}
                for q, i in live:
                    ht = hp.tile([128, 256], F16, name=f"h{q}", tag="h")
                    nc.scalar.activation(ht[:], phs[q][:],
                                         mybir.ActivationFunctionType.Silu)
                    hts[q] = ht
                pes = {}
                for q, i in live:
                    pes[q] = ps_e.tile([D, 128], F32, name=f"pe{q}", tag="pe")
                for q, i in live:
                    t = T_STEPS - 1 - i
                    nc.tensor.matmul(out=pes[q][:],
                                     lhsT=iax[:, t * D:(t + 1) * D],
                                     rhs=xcq[q][0:D, :], start=True, stop=False)
                for q, i in live:
                    nc.tensor.matmul(out=pes[q][:], lhsT=w2a,
                                     rhs=hts[q][:, 0:128],
                                     start=False, stop=False)
                for q, i in live:
                    nc.tensor.matmul(out=pes[q][:], lhsT=w2b,
                                     rhs=hts[q][:, 128:256],
                                     start=False, stop=True)
                for q, i in live:
                    t = T_STEPS - 1 - i
                    col = i * BL + q * 128
                    dst = (xcq[q][0:D, :] if i < T_STEPS - 1
                           else xout[:, q * 128:(q + 1) * 128])
                    nc.vector.scalar_tensor_tensor(
                        out=dst, in0=pes[q][:],
                        scalar=-float(C[t]), in1=nz[:, col:col + 128],
                        op0=mybir.AluOpType.mult, op1=mybir.AluOpType.add)

            nc.sync.dma_start(outT_d[:], xout[:])

    nc.compile()
    return nc


_CACHE = {}


def _get_program(G, consts):
    key = tuple(G.reshape(-1).tolist())
    if key not in _CACHE:
        _CACHE[key] = build_program(G, consts)
    return _CACHE[key]


def kernel(**inputs):
    per_core, G, consts = host_prep(inputs)
    nc = _get_program(G, consts)
    in_maps = [core for core, _ in per_core]
    res = run_bass_kernel_spmd(nc, in_maps, list(range(NCORES)))
    out = np.zeros((B, D), np.float32)
    for n in range(NCORES):
        _, rws = per_core[n]
        out[rws] = np.asarray(res.results[n]["outT"]).astype(np.float32).T
    return out



# revision 59
# speedup vs baseline: 1.0036x; 1.0021x over previous
"""Trainium2 Bass kernel for nn_DDPMVAEQueryEncoder.

Strategy (data-parallel over batch, 8 cores):
  * Host: bucket/pack rows into 4 bands of 1024 (fattest band first) to
    minimize gather padding; build int16 gather-index tiles; fold all
    weight-only matmuls; fold timestep embeddings into the x-state
    (x~ = x + temb_t) with per-step corrections folded into the noise
    tensor; precompute 1/sqrt(nnz) per row.
  * Device per core (512 batch rows):
      phase 1: bf16 embedding table with 256B row pitch gathered via
        128B-element dma_gather (one descriptor per lookup at half the
        256B-descriptor cost), bf16 pairwise tree-reduce on DVE, scale by
        1/sqrt(nnz), PE-transpose, one matmul per chunk for c^T.
      phase 2: 50 ancestral DDPM steps over FOUR independent 128-column
        chains (one per chunk) in fp16 to hide the per-step serial
        latency: ph = w1s^T @ [x~; c] (2 matmuls into one PSUM tile), one
        silu [128,256] on ACT, eps-psum via 4 matmuls (A_t x~,
        sigma-folded noise, W2^T h halves), x-update on DVE:
        x~' = (pe + temb'/(-C_t)) * (-C_t).
  * Host: un-permute rows, emit [4096, 64].
"""
import sys

import numpy as np

if "/opt/trn_rl_repo" not in sys.path:
    sys.path.insert(0, "/opt/trn_rl_repo")

import ml_dtypes
import concourse.bass as bass
import concourse.mybir as mybir
import concourse.tile as tile
from concourse.tile_rust import add_dep_helper
from concourse import bacc
from concourse import ap_utils
from concourse.bass import MemorySpace, round_up_to_multiple
from concourse.bass_utils import run_bass_kernel_spmd
from concourse.masks import make_identity

F32 = mybir.dt.float32
F32R = mybir.dt.float32r
F16 = mybir.dt.float16
BF16 = mybir.dt.bfloat16
FP8 = mybir.dt.float8e4
I16 = mybir.dt.int16

T_STEPS = 50
D = 64
B = 4096
L = 200
V = 100000
NCORES = 8
BL = B // NCORES          # 512 rows per core
NCHUNK = BL // 128        # 4 chunks of 128 rows = 4 scan chains
NSEG = 4
SEG = 25000               # index range per segment
SEGR = SEG + 1            # +1 zero row


def _schedule_consts():
    steps = T_STEPS
    scale = 1000.0 / steps
    betas = np.linspace(scale * 1e-4, scale * 2e-2, steps, dtype=np.float64)
    alphas = 1.0 - betas
    acp = np.cumprod(alphas)
    acp_prev = np.append(1.0, acp[:-1])
    sqrt_recip = np.sqrt(1.0 / acp)
    sqrt_recipm1 = np.sqrt(1.0 / acp - 1.0)
    post_var = betas * (1.0 - acp_prev) / (1.0 - acp)
    post_logvar = np.log(np.append(post_var[1], post_var[1:]))
    coef1 = betas * np.sqrt(acp_prev) / (1.0 - acp)
    coef2 = (1.0 - acp_prev) * np.sqrt(alphas) / (1.0 - acp)
    A = coef1 * sqrt_recip + coef2
    C = coef1 * sqrt_recipm1
    S = np.exp(0.5 * post_logvar)
    S[0] = 0.0
    return A, C, S


def _timestep_emb(Wt, bt):
    half = D // 2
    freqs = np.exp(-np.log(10000.0) * np.arange(half, dtype=np.float64) / half)
    t = np.arange(T_STEPS, dtype=np.float64)
    args = t[:, None] * freqs[None, :]
    temb = np.concatenate([np.cos(args), np.sin(args)], axis=-1)
    return temb.astype(np.float32) @ Wt + bt  # [50, 64] (temb_t = row t)


def host_prep(inputs):
    seq = np.asarray(inputs["seq"]).astype(np.int64)
    item_emb = np.asarray(inputs["item_emb"], dtype=np.float32)
    W_enc = np.asarray(inputs["W_enc"], dtype=np.float32)
    b_enc = np.asarray(inputs["b_enc"], dtype=np.float32)
    Wt = np.asarray(inputs["Wt"], dtype=np.float32)
    bt = np.asarray(inputs["bt"], dtype=np.float32)
    Wc = np.asarray(inputs["Wc"], dtype=np.float32)
    bc = np.asarray(inputs["bc"], dtype=np.float32)
    W1 = np.asarray(inputs["W1"], dtype=np.float32)
    b1 = np.asarray(inputs["b1"], dtype=np.float32)
    W2 = np.asarray(inputs["W2"], dtype=np.float32)
    b2 = np.asarray(inputs["b2"], dtype=np.float32)
    init_noise = np.asarray(inputs["init_noise"], dtype=np.float32)
    step_noise = np.asarray(inputs["step_noise"], dtype=np.float32)

    assert np.abs(b1).max() == 0.0, "b1 must be zero (silu bias is folded out)"

    A, C, S = _schedule_consts()
    temb = _timestep_emb(Wt, bt).astype(np.float64)  # [50, 64]

    # ---- row packing: greedy bands minimizing per-band per-range max counts;
    # fattest band FIRST so the last chunk (shortest gathers) gates the scan.
    bucket = seq // SEG
    counts = np.stack([(bucket == k).sum(1) for k in range(NSEG)], 1)
    mx = counts.max(1)
    idx_desc = np.argsort(-mx, kind="stable")
    bands = [[] for _ in range(NCHUNK)]
    bmax = np.zeros((NCHUNK, NSEG), np.int64)
    for r in idx_desc:
        best, bestcost = None, None
        for b in range(NCHUNK):
            if len(bands[b]) >= NCORES * 128:
                continue
            cost = np.maximum(bmax[b], counts[r]).sum() - bmax[b].sum()
            if bestcost is None or cost < bestcost:
                best, bestcost = b, cost
        bands[best].append(r)
        bmax[best] = np.maximum(bmax[best], counts[r])
    border = np.argsort(-bmax.sum(1), kind="stable")   # fattest first
    order = np.concatenate([np.array(bands[b]) for b in border])
    rows = order.reshape(NCHUNK, NCORES, 128)          # [chunk, core, row]

    # fp8 table, 256B row pitch (cols 64:256 zero), +1 zero row per segment
    tbl = np.zeros((NSEG * SEGR, 256), ml_dtypes.float8_e4m3fn)
    for k in range(NSEG):
        tbl[k * SEGR: k * SEGR + SEG, 0:D] = item_emb[k * SEG: (k + 1) * SEG]

    G = counts[order].reshape(NCHUNK, NCORES * 128, NSEG).max(1)
    G = np.maximum(G, 1).astype(np.int64)              # [chunk, 4]

    # int16 gather index tiles per (core, chunk, range)
    idx16 = [[[None] * NSEG for _ in range(NCHUNK)] for _ in range(NCORES)]
    for c in range(NCHUNK):
        for n in range(NCORES):
            rs = rows[c, n]
            sq = seq[rs]
            bk = bucket[rs]
            for k in range(NSEG):
                g = int(G[c, k])
                val = np.full((128, g), SEG, np.int16)
                for p in range(128):
                    e = sq[p][bk[p] == k] - k * SEG
                    val[p, : len(e)] = e.astype(np.int16)
                # slot i = gg*128 + p  ->  idx tile [i%16, i//16]
                v = val.reshape(8, 16, g)              # [p//16, p%16, g]
                arr = np.transpose(v, (1, 2, 0)).reshape(16, g * 8)
                idx16[n][c][k] = np.ascontiguousarray(np.tile(arr, (8, 1)))

    wec = (W_enc[:, :D] @ Wc).astype(np.float32)
    bec = (b_enc[:D] @ Wc + bc).astype(np.float32).reshape(D, 1)
    # f16 const bundle [128, 384]: w1s | w2a | w2b
    cb16 = np.zeros((128, 384), np.float16)
    cb16[:, 0:256] = np.vstack([W1, W1])
    cb16[:, 256:320] = W2[0:128, :]
    cb16[:, 320:384] = W2[128:256, :]

    # per-step diagonal fold coefficients (built into diag blocks on-device)
    iaxc = np.zeros((D, T_STEPS), np.float32)
    iaxeff = np.empty(T_STEPS, np.float64)
    for t in range(T_STEPS):
        rat = np.float32(A[t] / (-C[t]))
        iaxc[:, t] = rat
        iaxeff[t] = np.float64(np.float16(rat))   # f16 diag as built
    Aeff = iaxeff * (-C)   # effective x passthrough after f16 rounding

    # noise+temb fold, feature-major per step i (t = 49-i):
    # x~' = (-C_t)*pe + nzf_i with
    # nzf_i = -Aeff_t*temb_t - C_t*b2 + S_t*n_i^T + temb_{t-1} (0 at t=0)
    per_core = []
    for n in range(NCORES):
        rws = rows[:, n, :].reshape(-1)
        nT = np.empty((T_STEPS, D, BL), np.float64)
        for i in range(T_STEPS):
            t = T_STEPS - 1 - i
            base = -Aeff[t] * temb[t] - C[t] * b2.astype(np.float64)
            if t > 0:
                base = base + temb[t - 1]
            nT[i] = base[:, None] + S[t] * step_noise[i][rws].T.astype(np.float64)
        noiseT = np.ascontiguousarray(
            nT.transpose(1, 0, 2).reshape(D, T_STEPS * BL)).astype(np.float16)
        x0T = np.ascontiguousarray(
            (init_noise[rws] + temb[T_STEPS - 1][None, :]).T).astype(np.float16)
        nnz = np.count_nonzero(seq[rws], axis=1).astype(np.float64)
        rsq = (1.0 / np.sqrt(np.maximum(nnz, 1.0))).astype(np.float32)
        rsqt = np.ascontiguousarray(rsq.reshape(NCHUNK, 128).T)   # [128, NCHUNK]
        # f32 const bundle [128, 119]: wec | bec | rsq | iaxc
        cb32 = np.zeros((128, 119), np.float32)
        cb32[0:D, 0:64] = wec
        cb32[0:D, 64:65] = bec
        cb32[:, 65:69] = rsqt
        cb32[0:D, 69:119] = iaxc
        # merged const bundle, f16-typed: [cb16 | cb32 viewed as f16]
        cb = np.concatenate([cb16, cb32.view(np.float16)], axis=1)
        core = dict(tbl=tbl, noiseT=noiseT, x0T=x0T, cb=np.ascontiguousarray(cb))
        for c in range(NCHUNK):
            # concat in gather order (fattest segment first)
            ks = sorted(range(NSEG), key=lambda k: -G[c, k])
            parts = [idx16[n][c][k] for k in ks]
            core[f"idxc_{c}"] = np.ascontiguousarray(np.concatenate(parts, 1))
        per_core.append((core, rws))

    consts = dict(A=A.astype(np.float32), C=C.astype(np.float32))
    return per_core, G, consts


def dma_gather_small(gp, out_ap, in_ap, idxs_ap, num_idxs, num_idxs_reg,
                     elem_size, elem_step, single_packet=False, queue_num=0):
    """nc.gpsimd.dma_gather without the elem_size_bytes%256 assert
    (transpose=False, DRAM source). elem_step*dtype must be %256."""
    assert idxs_ap.dtype == mybir.dt.int16
    assert in_ap.space == MemorySpace.DRAM
    assert idxs_ap.space == MemorySpace.SBUF
    assert out_ap.space == MemorySpace.SBUF
    assert ap_utils.ap_is_contiguous(out_ap.ap[1:])
    assert ap_utils.ap_is_contiguous(idxs_ap.ap[1:])
    assert in_ap.ap[-1][1] == out_ap.ap[-1][1] == elem_size
    assert out_ap.ap[0][1] * out_ap.ap[1][1] == round_up_to_multiple(num_idxs, 128)
    assert in_ap.ap[0][0] == elem_step
    stride_bytes = elem_step * mybir.dt.size(in_ap.dtype)
    assert stride_bytes % 256 == 0 and stride_bytes // 256 < 256
    _in_ap = gp.lower_ap_dma(in_ap, for_custom_bir_dma=True)
    _idxs_ap = gp.lower_ap(idxs_ap)
    _out_ap = gp.lower_ap(out_ap)
    return gp.add_instruction(
        mybir.InstDMAGatherAnt(
            name=gp.bass.get_next_instruction_name(),
            ins=[*_in_ap, _idxs_ap, gp.lower_val_access(gp.to_reg(num_idxs_reg))],
            outs=[_out_ap],
            transpose=False,
            num_idxs=num_idxs,
            elem_size=elem_size,
            stride_bytes_256=stride_bytes // 256,
            gen_mode=0,
            single_packet=single_packet,
            queue_num=queue_num,
            sbuf_tokens_per_rank=0,
            sbuf_free_dim_per_rank=0,
            sbuf_free_dim_pad_per_rank=0,
            sbuf_byte_offset=0,
        )
    )


def build_program(G, consts, N_WARM=55, NZ_PIECES=5):
    A, C = consts["A"], consts["C"]
    nc = bacc.Bacc("TRN2", target_bir_lowering=False, debug=False,
                   num_devices=NCORES)

    din = lambda name, shape, dt=F32: nc.dram_tensor(
        name, shape, dt, kind="ExternalInput").ap()
    tbl_d = din("tbl", [NSEG * SEGR, 256], FP8)
    noiseT_d = din("noiseT", [D, T_STEPS * BL], F16)
    x0T_d = din("x0T", [D, BL], F16)
    cb_d = din("cb", [128, 384 + 238], F16)
    idx_d = {}
    for c in range(NCHUNK):
        idx_d[c] = din(f"idxc_{c}", [128, 8 * int(G[c].sum())], I16)
    outT_d = nc.dram_tensor("outT", [D, BL], F16, kind="ExternalOutput").ap()

    Gmax = int(G.max())

    with tile.TileContext(nc) as tc:
        with (
            tc.tile_pool(name="const", bufs=1) as constp,
            tc.tile_pool(name="gidx", bufs=1) as gidxp,
            tc.tile_pool(name="gdst", bufs=5) as gdstp,
            tc.tile_pool(name="redb", bufs=4) as redb,
            tc.tile_pool(name="redp", bufs=6) as redp,
            tc.tile_pool(name="xcp", bufs=1) as xcp,
            tc.tile_pool(name="hp", bufs=6) as hp,
            tc.tile_pool(name="ps_t", bufs=1, space="PSUM") as ps_t,
            tc.tile_pool(name="ps_h", bufs=3, space="PSUM") as ps_h,
            tc.tile_pool(name="ps_e", bufs=4, space="PSUM") as ps_e,
        ):
            # ---- bundled consts (tile now, DMA issued after the idx loads)
            cbt = constp.tile([128, 384 + 238], F16, name="cbt")
            ident = constp.tile([128, 128], F32, name="ident")
            make_identity(nc, ident[:])
            w1s = cbt[:, 0:256]
            w2a = cbt[:, 256:320]
            w2b = cbt[:, 320:384]
            cb32 = cbt[:, 384:622].bitcast(F32)
            wec = cb32[0:D, 0:64]
            bec = cb32[0:D, 64:65]
            rsq = cb32[:, 65:69]
            iaxc = cb32[0:D, 69:119]

            # on-device diag blocks: iax (f16), per-chunk rsq diag (f32)
            # (tiles allocated here; ops emitted after the cbt DMA below)
            iax = constp.tile([D, T_STEPS * D], F16, name="iax")
            rsqd = [constp.tile([128, 128], F32, name=f"rsqd{c}")
                    for c in range(NCHUNK)]

            diag_jobs = []

            def build_diags():
                for c in range(NCHUNK):
                    diag_jobs.append(lambda c=c: nc.vector.tensor_scalar(
                        out=rsqd[c][:], in0=ident[:], scalar1=rsq[:, c:c + 1],
                        scalar2=None, op0=mybir.AluOpType.mult))
                for t in range(T_STEPS):
                    # on ACT (idle in the gather window; DVE is reduce-bound)
                    diag_jobs.append(lambda t=t: nc.scalar.activation(
                        iax[:, t * D:(t + 1) * D], ident[0:D, 0:D],
                        mybir.ActivationFunctionType.Identity,
                        scale=iaxc[:, t:t + 1]))

            def emit_diags(n):
                while n > 0 and diag_jobs:
                    diag_jobs.pop(0)()
                    n -= 1

            nz = constp.tile([D, T_STEPS * BL], F16, name="nz")
            xout = constp.tile([D, BL], F16, name="xout")
            xcq = [xcp.tile([128, 128], F16, name=f"xc{q}", tag=f"xc{q}")
                   for q in range(NCHUNK)]
            poolT = [constp.tile([D, 128], F32, name=f"poolT{q}")
                     for q in range(NCHUNK)]

            idx_t = {}
            # per-chunk idx col offset for segment k (gather order = G desc)
            idx_off = {}
            for c in range(NCHUNK):
                ks = sorted(range(NSEG), key=lambda k: -G[c, k])
                off = 0
                for k in ks:
                    idx_off[(c, k)] = off
                    off += 8 * int(G[c, k])

            def load_idx(c, split_first=0):
                gs = int(G[c].sum())
                if split_first:
                    # first gather's idx in its own tile, loaded first
                    s = 8 * split_first
                    ita = gidxp.tile([128, s], I16, name=f"it{c}a", tag=f"it{c}a")
                    nc.sync.dma_start(ita[:], idx_d[c][:, 0:s])
                    it = gidxp.tile([128, 8 * gs - s], I16, name=f"it{c}",
                                    tag=f"it{c}")
                    nc.sync.dma_start(it[:], idx_d[c][:, s:])
                    idx_t[c] = (ita, it, s)
                else:
                    it = gidxp.tile([128, 8 * gs], I16, name=f"it{c}",
                                    tag=f"it{c}")
                    nc.sync.dma_start(it[:], idx_d[c][:])
                    idx_t[c] = (None, it, 0)

            def idx_ap(c, off, width):
                ita, it, s = idx_t[c]
                if ita is not None and off < s:
                    assert off + width <= s
                    return ita[:, off:off + width]
                return it[:, off - s:off - s + width]

            def do_gather(c, k, soff, g):
                off = idx_off[(c, k)] + 8 * soff
                dst = gdstp.tile([128, Gmax * D], FP8, name="dst", tag="dst")
                return dst, dma_gather_small(
                    nc.gpsimd,
                    dst[:, : g * D].rearrange("p (g d) -> p g d", g=g, d=D),
                    tbl_d[k * SEGR:(k + 1) * SEGR, 0:D],
                    idx_ap(c, off, 8 * g), 128 * g, 128 * g, D, 256)

            def do_reduce(g, dst, acc):
                """fp8 pair-add into bf16, bf16 tree to 2, mixed-add to f32."""
                ops = []
                m = g // 2
                if m == 0:
                    sk = redp.tile([128, D], F32, name="sk", tag="rk")
                    ops.append(nc.vector.tensor_copy(sk[:], dst[:, 0:D]))
                else:
                    red = redb.tile([128, (Gmax // 2 + 1) * D], BF16,
                                    name="red", tag="red")
                    op = nc.vector.tensor_tensor(
                        out=red[:, : m * D], in0=dst[:, : m * D],
                        in1=dst[:, m * D: 2 * m * D], op=mybir.AluOpType.add)
                    ops.append(op)
                    w = m
                    if g % 2:
                        ops.append(nc.vector.tensor_copy(
                            red[:, m * D:(m + 1) * D], dst[:, (g - 1) * D:g * D]))
                        w = m + 1
                    while w > 2:
                        mm2 = w // 2
                        ops.append(nc.vector.tensor_tensor(
                            out=red[:, : mm2 * D], in0=red[:, : mm2 * D],
                            in1=red[:, (w - mm2) * D: w * D],
                            op=mybir.AluOpType.add))
                        w = w - mm2
                    sk = redp.tile([128, D], F32, name="sk", tag="rk")
                    if w == 2:
                        ops.append(nc.vector.tensor_tensor(
                            out=sk[:], in0=red[:, 0:D], in1=red[:, D:2 * D],
                            op=mybir.AluOpType.add))
                    else:
                        ops.append(nc.vector.tensor_copy(sk[:], red[:, 0:D]))
                if acc is None:
                    return sk, ops
                acc2 = redp.tile([128, D], F32, name="acc2", tag="rk")
                ops.append(nc.vector.tensor_tensor(
                    out=acc2[:], in0=acc[:], in1=sk[:], op=mybir.AluOpType.add))
                return acc2, ops

            def do_finish_chunk(c, acc):
                # transpose + rsq fold in one regular matmul:
                # pt = acc.T @ diag(rsq_c)
                pt = ps_t.tile([D, 128], F32, name="pt", tag="pt")
                nc.tensor.matmul(out=pt[:], lhsT=acc[:], rhs=rsqd[c][:],
                                 start=True, stop=True)
                nc.scalar.copy(poolT[c][:], pt[:])
                # conditioning for chain c
                pc = ps_t.tile([D, 128], F32, name="pc", tag="pt")
                nc.tensor.matmul(out=pc[:], lhsT=wec, rhs=poolT[c][:],
                                 start=True, stop=True)
                nc.scalar.activation(xcq[c][D:128, :], pc[:],
                                     mybir.ActivationFunctionType.Identity,
                                     bias=bec)

            # ---- phase 1: gathers + reduces, pipelined; within each chunk
            # the fattest segment first (leanest last => shortest tail).
            # The very first gather is split in two so its descriptor-gen
            # overlaps its own transfer.
            jobs = []
            for c in range(NCHUNK):
                ks = sorted(range(NSEG), key=lambda k: -G[c, k])
                jobs += [(c, k, 0, int(G[c, k])) for k in ks]
            # split the FIRST job: a small leading sub-gather primes the
            # DMA pipe earlier (short descgen before the first transfer)
            c0, k0, _, g0 = jobs[0]
            SPLIT0 = 12
            jobs[0:1] = [(c0, k0, 0, SPLIT0), (c0, k0, SPLIT0, g0 - SPLIT0)]
            # split the last job so the final reduce tail is shorter
            cl, kl, _, gl = jobs[-1]
            jobs[-1:] = [(cl, kl, 0, gl // 2), (cl, kl, gl // 2, gl - gl // 2)]
            left = {c: sum(1 for jb in jobs if jb[0] == c) for c in range(NCHUNK)}
            PIPE_G = 4
            load_idx(0, split_first=int(G[0, jobs[0][1]]))
            load_idx(1)
            nc.sync.dma_start(cbt[:], cb_d[:])
            build_diags()
            for q in range(NCHUNK):
                nc.sync.dma_start(xcq[q][0:D, :], x0T_d[:, q * 128:(q + 1) * 128])
            gdsts = {}
            for j in range(PIPE_G):
                gdsts[j] = do_gather(*jobs[j])
            accs = {c: None for c in range(NCHUNK)}
            warm_dep = None
            last_gather = None
            loaded = {0, 1}
            for j in range(len(jobs)):
                c, k, soff, g = jobs[j]
                if j + PIPE_G < len(jobs):
                    cn = jobs[j + PIPE_G][0]
                    if cn not in loaded:
                        load_idx(cn)
                        loaded.add(cn)
                    gdsts[j + PIPE_G] = do_gather(*jobs[j + PIPE_G])
                dst, ginst = gdsts.pop(j)
                if j == len(jobs) - 1:
                    last_gather = ginst
                accs[c], ops = do_reduce(g, dst, accs[c])
                if j == len(jobs) - 1:
                    final_red = ops[-1]
                emit_diags(4)
                if j == len(jobs) - 2:
                    warm_dep = ops[0]
                left[c] -= 1
                if left[c] == 0:
                    if j == len(jobs) - 1:
                        # PE warm-up BEFORE the last chunk-finish so it runs
                        # during the final gather/reduce, not after pc3 (the
                        # in-order PE queue would put it on the critical path)
                        warm_t = ps_t.tile([D, 128], F32, name="warm_t",
                                           tag="pt")
                        for i in range(N_WARM):
                            wm = nc.tensor.matmul(out=warm_t[:], lhsT=w2a,
                                                  rhs=w1s[:, 0:128],
                                                  start=True, stop=True)
                            if i == 0 and warm_dep is not None:
                                add_dep_helper(wm.ins, warm_dep.ins, sync=False,
                                               reason="warm near last reduce")
                    do_finish_chunk(c, accs[c])

            # ---- deferred DMAs (gated behind the last gather)
            def gated_dma(dst_ap, src_ap):
                inst = nc.sync.dma_start(dst_ap, src_ap)
                add_dep_helper(inst.ins, last_gather.ins, sync=True,
                               reason="defer until gathers done")
                return inst

            npc = T_STEPS // NZ_PIECES
            for p in range(NZ_PIECES):
                gated_dma(nz[:, p * npc * BL:(p + 1) * npc * BL],
                          noiseT_d[:, p * npc * BL:(p + 1) * npc * BL])

            # ---- phase 2: 50 steps, four 128-col chains in lockstep.
            # Matmuls grouped by stationary weight (5 LdWeights per wave);
            # noise+temb folded into the DVE x-update.
            for k in range(T_STEPS):
                live = [(q, k) for q in range(NCHUNK)]
                phs = {}
                for q, i in live:
                    phs[q] = ps_h.tile([128, 256], F32, name=f"ph{q}",
                                       tag="ph")
                for q, i in live:
                    nc.tensor.matmul(out=phs[q][:, 0:128], lhsT=w1s[:, 0:128],
                                     rhs=xcq[q][:], start=True, stop=True)
                for q, i in live:
                    nc.tensor.matmul(out=phs[q][:, 128:256],
                                     lhsT=w1s[:, 128:256],
                                     rhs=xcq[q][:], start=True, stop=True)
                hts = {}
                for q, i in live:
                    ht = hp.tile([128, 256], F16, name=f"h{q}", tag="h")
                    nc.scalar.activation(ht[:], phs[q][:],
                                         mybir.ActivationFunctionType.Silu)
                    hts[q] = ht
                pes = {}
                for q, i in live:
                    pes[q] = ps_e.tile([D, 128], F32, name=f"pe{q}", tag="pe")
                for q, i in live:
                    t = T_STEPS - 1 - i
                    nc.tensor.matmul(out=pes[q][:],
                                     lhsT=iax[:, t * D:(t + 1) * D],
                                     rhs=xcq[q][0:D, :], start=True, stop=False)
                for q, i in live:
                    nc.tensor.matmul(out=pes[q][:], lhsT=w2a,
                                     rhs=hts[q][:, 0:128],
                                     start=False, stop=False)
                for q, i in live:
                    nc.tensor.matmul(out=pes[q][:], lhsT=w2b,
                                     rhs=hts[q][:, 128:256],
                                     start=False, stop=True)
                for q, i in live:
                    t = T_STEPS - 1 - i
                    col = i * BL + q * 128
                    dst = (xcq[q][0:D, :] if i < T_STEPS - 1
                           else xout[:, q * 128:(q + 1) * 128])
                    stt = nc.vector.scalar_tensor_tensor(
                        out=dst, in0=pes[q][:],
                        scalar=-float(C[t]), in1=nz[:, col:col + 128],
                        op0=mybir.AluOpType.mult, op1=mybir.AluOpType.add)
                    if i == 0:
                        # keep the step-0 x-updates BEHIND the final reduce
                        # in the DVE queue: they stall on the noise load, and
                        # scheduled ahead they head-block the last chunk's
                        # reduce -> conditioning -> the whole critical chain
                        add_dep_helper(stt.ins, final_red.ins, sync=False,
                                       reason="step-0 upd after final reduce")

            nc.sync.dma_start(outT_d[:], xout[:])

    nc.compile()
    return nc


_CACHE = {}


def _get_program(G, consts):
    key = tuple(G.reshape(-1).tolist())
    if key not in _CACHE:
        _CACHE[key] = build_program(G, consts)
    return _CACHE[key]


def kernel(**inputs):
    per_core, G, consts = host_prep(inputs)
    nc = _get_program(G, consts)
    in_maps = [core for core, _ in per_core]
    res = run_bass_kernel_spmd(nc, in_maps, list(range(NCORES)))
    out = np.zeros((B, D), np.float32)
    for n in range(NCORES):
        _, rws = per_core[n]
        out[rws] = np.asarray(res.results[n]["outT"]).astype(np.float32).T
    return out



# revision 63
# speedup vs baseline: 1.0039x; 1.0003x over previous
"""Trainium2 Bass kernel for nn_DDPMVAEQueryEncoder.

Strategy (data-parallel over batch, 8 cores):
  * Host: bucket/pack rows into 4 bands of 1024 (fattest band first) to
    minimize gather padding; build int16 gather-index tiles; fold all
    weight-only matmuls; fold timestep embeddings into the x-state
    (x~ = x + temb_t) with per-step corrections folded into the noise
    tensor; precompute 1/sqrt(nnz) per row.
  * Device per core (512 batch rows):
      phase 1: bf16 embedding table with 256B row pitch gathered via
        128B-element dma_gather (one descriptor per lookup at half the
        256B-descriptor cost), bf16 pairwise tree-reduce on DVE, scale by
        1/sqrt(nnz), PE-transpose, one matmul per chunk for c^T.
      phase 2: 50 ancestral DDPM steps over FOUR independent 128-column
        chains (one per chunk) in fp16 to hide the per-step serial
        latency: ph = w1s^T @ [x~; c] (2 matmuls into one PSUM tile), one
        silu [128,256] on ACT, eps-psum via 4 matmuls (A_t x~,
        sigma-folded noise, W2^T h halves), x-update on DVE:
        x~' = (pe + temb'/(-C_t)) * (-C_t).
  * Host: un-permute rows, emit [4096, 64].
"""
import sys

import numpy as np

if "/opt/trn_rl_repo" not in sys.path:
    sys.path.insert(0, "/opt/trn_rl_repo")

import ml_dtypes
import concourse.bass as bass
import concourse.mybir as mybir
import concourse.tile as tile
from concourse.tile_rust import add_dep_helper
from concourse import bacc
from concourse import ap_utils
from concourse.bass import MemorySpace, round_up_to_multiple
from concourse.bass_utils import run_bass_kernel_spmd
from concourse.masks import make_identity

F32 = mybir.dt.float32
F32R = mybir.dt.float32r
F16 = mybir.dt.float16
BF16 = mybir.dt.bfloat16
FP8 = mybir.dt.float8e4
I16 = mybir.dt.int16

T_STEPS = 50
D = 64
B = 4096
L = 200
V = 100000
NCORES = 8
BL = B // NCORES          # 512 rows per core
NCHUNK = BL // 128        # 4 chunks of 128 rows = 4 scan chains
NSEG = 4
SEG = 25000               # index range per segment
SEGR = SEG + 1            # +1 zero row


def _schedule_consts():
    steps = T_STEPS
    scale = 1000.0 / steps
    betas = np.linspace(scale * 1e-4, scale * 2e-2, steps, dtype=np.float64)
    alphas = 1.0 - betas
    acp = np.cumprod(alphas)
    acp_prev = np.append(1.0, acp[:-1])
    sqrt_recip = np.sqrt(1.0 / acp)
    sqrt_recipm1 = np.sqrt(1.0 / acp - 1.0)
    post_var = betas * (1.0 - acp_prev) / (1.0 - acp)
    post_logvar = np.log(np.append(post_var[1], post_var[1:]))
    coef1 = betas * np.sqrt(acp_prev) / (1.0 - acp)
    coef2 = (1.0 - acp_prev) * np.sqrt(alphas) / (1.0 - acp)
    A = coef1 * sqrt_recip + coef2
    C = coef1 * sqrt_recipm1
    S = np.exp(0.5 * post_logvar)
    S[0] = 0.0
    return A, C, S


def _timestep_emb(Wt, bt):
    half = D // 2
    freqs = np.exp(-np.log(10000.0) * np.arange(half, dtype=np.float64) / half)
    t = np.arange(T_STEPS, dtype=np.float64)
    args = t[:, None] * freqs[None, :]
    temb = np.concatenate([np.cos(args), np.sin(args)], axis=-1)
    return temb.astype(np.float32) @ Wt + bt  # [50, 64] (temb_t = row t)


def host_prep(inputs):
    seq = np.asarray(inputs["seq"]).astype(np.int64)
    item_emb = np.asarray(inputs["item_emb"], dtype=np.float32)
    W_enc = np.asarray(inputs["W_enc"], dtype=np.float32)
    b_enc = np.asarray(inputs["b_enc"], dtype=np.float32)
    Wt = np.asarray(inputs["Wt"], dtype=np.float32)
    bt = np.asarray(inputs["bt"], dtype=np.float32)
    Wc = np.asarray(inputs["Wc"], dtype=np.float32)
    bc = np.asarray(inputs["bc"], dtype=np.float32)
    W1 = np.asarray(inputs["W1"], dtype=np.float32)
    b1 = np.asarray(inputs["b1"], dtype=np.float32)
    W2 = np.asarray(inputs["W2"], dtype=np.float32)
    b2 = np.asarray(inputs["b2"], dtype=np.float32)
    init_noise = np.asarray(inputs["init_noise"], dtype=np.float32)
    step_noise = np.asarray(inputs["step_noise"], dtype=np.float32)

    assert np.abs(b1).max() == 0.0, "b1 must be zero (silu bias is folded out)"

    A, C, S = _schedule_consts()
    temb = _timestep_emb(Wt, bt).astype(np.float64)  # [50, 64]

    # ---- row packing: greedy bands minimizing per-band per-range max counts;
    # fattest band FIRST so the last chunk (shortest gathers) gates the scan.
    bucket = seq // SEG
    counts = np.stack([(bucket == k).sum(1) for k in range(NSEG)], 1)
    mx = counts.max(1)
    idx_desc = np.argsort(-mx, kind="stable")
    bands = [[] for _ in range(NCHUNK)]
    bmax = np.zeros((NCHUNK, NSEG), np.int64)
    for r in idx_desc:
        best, bestcost = None, None
        for b in range(NCHUNK):
            if len(bands[b]) >= NCORES * 128:
                continue
            cost = np.maximum(bmax[b], counts[r]).sum() - bmax[b].sum()
            if bestcost is None or cost < bestcost:
                best, bestcost = b, cost
        bands[best].append(r)
        bmax[best] = np.maximum(bmax[best], counts[r])
    border = np.argsort(-bmax.sum(1), kind="stable")   # fattest first
    order = np.concatenate([np.array(bands[b]) for b in border])
    rows = order.reshape(NCHUNK, NCORES, 128)          # [chunk, core, row]

    # fp8 table, 256B row pitch (cols 64:256 zero), +1 zero row per segment
    tbl = np.zeros((NSEG * SEGR, 256), ml_dtypes.float8_e4m3fn)
    for k in range(NSEG):
        tbl[k * SEGR: k * SEGR + SEG, 0:D] = item_emb[k * SEG: (k + 1) * SEG]

    G = counts[order].reshape(NCHUNK, NCORES * 128, NSEG).max(1)
    G = np.maximum(G, 1).astype(np.int64)              # [chunk, 4]

    # int16 gather index tiles per (core, chunk, range)
    idx16 = [[[None] * NSEG for _ in range(NCHUNK)] for _ in range(NCORES)]
    for c in range(NCHUNK):
        for n in range(NCORES):
            rs = rows[c, n]
            sq = seq[rs]
            bk = bucket[rs]
            for k in range(NSEG):
                g = int(G[c, k])
                val = np.full((128, g), SEG, np.int16)
                for p in range(128):
                    e = sq[p][bk[p] == k] - k * SEG
                    val[p, : len(e)] = e.astype(np.int16)
                # slot i = gg*128 + p  ->  idx tile [i%16, i//16]
                v = val.reshape(8, 16, g)              # [p//16, p%16, g]
                arr = np.transpose(v, (1, 2, 0)).reshape(16, g * 8)
                idx16[n][c][k] = np.ascontiguousarray(np.tile(arr, (8, 1)))

    wec = (W_enc[:, :D] @ Wc).astype(np.float32)
    bec = (b_enc[:D] @ Wc + bc).astype(np.float32).reshape(D, 1)
    # f16 const bundle [128, 384]: w1s | w2a | w2b
    cb16 = np.zeros((128, 384), np.float16)
    cb16[:, 0:256] = np.vstack([W1, W1])
    cb16[:, 256:320] = W2[0:128, :]
    cb16[:, 320:384] = W2[128:256, :]

    # per-step diagonal fold coefficients (built into diag blocks on-device)
    iaxc = np.zeros((D, T_STEPS), np.float32)
    iaxeff = np.empty(T_STEPS, np.float64)
    for t in range(T_STEPS):
        rat = np.float32(A[t] / (-C[t]))
        iaxc[:, t] = rat
        iaxeff[t] = np.float64(np.float16(rat))   # f16 diag as built
    Aeff = iaxeff * (-C)   # effective x passthrough after f16 rounding

    # noise+temb fold, feature-major per step i (t = 49-i):
    # x~' = (-C_t)*pe + nzf_i with
    # nzf_i = -Aeff_t*temb_t - C_t*b2 + S_t*n_i^T + temb_{t-1} (0 at t=0)
    per_core = []
    for n in range(NCORES):
        rws = rows[:, n, :].reshape(-1)
        nT = np.empty((T_STEPS, D, BL), np.float64)
        for i in range(T_STEPS):
            t = T_STEPS - 1 - i
            base = -Aeff[t] * temb[t] - C[t] * b2.astype(np.float64)
            if t > 0:
                base = base + temb[t - 1]
            nT[i] = base[:, None] + S[t] * step_noise[i][rws].T.astype(np.float64)
        noiseT = np.ascontiguousarray(
            nT.transpose(1, 0, 2).reshape(D, T_STEPS * BL)).astype(np.float16)
        x0T = np.ascontiguousarray(
            (init_noise[rws] + temb[T_STEPS - 1][None, :]).T).astype(np.float16)
        nnz = np.count_nonzero(seq[rws], axis=1).astype(np.float64)
        rsq = (1.0 / np.sqrt(np.maximum(nnz, 1.0))).astype(np.float32)
        rsqt = np.ascontiguousarray(rsq.reshape(NCHUNK, 128).T)   # [128, NCHUNK]
        # f32 const bundle [128, 119]: wec | bec | rsq | iaxc
        cb32 = np.zeros((128, 119), np.float32)
        cb32[0:D, 0:64] = wec
        cb32[0:D, 64:65] = bec
        cb32[:, 65:69] = rsqt
        cb32[0:D, 69:119] = iaxc
        # merged const bundle, f16-typed: [cb16 | cb32 viewed as f16]
        cb = np.concatenate([cb16, cb32.view(np.float16)], axis=1)
        core = dict(tbl=tbl, noiseT=noiseT, x0T=x0T, cb=np.ascontiguousarray(cb))
        for c in range(NCHUNK):
            # concat in gather order (fattest segment first)
            ks = sorted(range(NSEG), key=lambda k: -G[c, k])
            parts = [idx16[n][c][k] for k in ks]
            core[f"idxc_{c}"] = np.ascontiguousarray(np.concatenate(parts, 1))
        per_core.append((core, rws))

    consts = dict(A=A.astype(np.float32), C=C.astype(np.float32))
    return per_core, G, consts


def dma_gather_small(gp, out_ap, in_ap, idxs_ap, num_idxs, num_idxs_reg,
                     elem_size, elem_step, single_packet=False, queue_num=0):
    """nc.gpsimd.dma_gather without the elem_size_bytes%256 assert
    (transpose=False, DRAM source). elem_step*dtype must be %256."""
    assert idxs_ap.dtype == mybir.dt.int16
    assert in_ap.space == MemorySpace.DRAM
    assert idxs_ap.space == MemorySpace.SBUF
    assert out_ap.space == MemorySpace.SBUF
    assert ap_utils.ap_is_contiguous(out_ap.ap[1:])
    assert ap_utils.ap_is_contiguous(idxs_ap.ap[1:])
    assert in_ap.ap[-1][1] == out_ap.ap[-1][1] == elem_size
    assert out_ap.ap[0][1] * out_ap.ap[1][1] == round_up_to_multiple(num_idxs, 128)
    assert in_ap.ap[0][0] == elem_step
    stride_bytes = elem_step * mybir.dt.size(in_ap.dtype)
    assert stride_bytes % 256 == 0 and stride_bytes // 256 < 256
    _in_ap = gp.lower_ap_dma(in_ap, for_custom_bir_dma=True)
    _idxs_ap = gp.lower_ap(idxs_ap)
    _out_ap = gp.lower_ap(out_ap)
    return gp.add_instruction(
        mybir.InstDMAGatherAnt(
            name=gp.bass.get_next_instruction_name(),
            ins=[*_in_ap, _idxs_ap, gp.lower_val_access(gp.to_reg(num_idxs_reg))],
            outs=[_out_ap],
            transpose=False,
            num_idxs=num_idxs,
            elem_size=elem_size,
            stride_bytes_256=stride_bytes // 256,
            gen_mode=0,
            single_packet=single_packet,
            queue_num=queue_num,
            sbuf_tokens_per_rank=0,
            sbuf_free_dim_per_rank=0,
            sbuf_free_dim_pad_per_rank=0,
            sbuf_byte_offset=0,
        )
    )


def build_program(G, consts, N_WARM=55, NZ_PIECES=5):
    A, C = consts["A"], consts["C"]
    nc = bacc.Bacc("TRN2", target_bir_lowering=False, debug=False,
                   num_devices=NCORES)

    din = lambda name, shape, dt=F32: nc.dram_tensor(
        name, shape, dt, kind="ExternalInput").ap()
    tbl_d = din("tbl", [NSEG * SEGR, 256], FP8)
    noiseT_d = din("noiseT", [D, T_STEPS * BL], F16)
    x0T_d = din("x0T", [D, BL], F16)
    cb_d = din("cb", [128, 384 + 238], F16)
    idx_d = {}
    for c in range(NCHUNK):
        idx_d[c] = din(f"idxc_{c}", [128, 8 * int(G[c].sum())], I16)
    outT_d = nc.dram_tensor("outT", [D, BL], F16, kind="ExternalOutput").ap()

    Gmax = int(G.max())

    with tile.TileContext(nc) as tc:
        with (
            tc.tile_pool(name="const", bufs=1) as constp,
            tc.tile_pool(name="gidx", bufs=1) as gidxp,
            tc.tile_pool(name="gdst", bufs=5) as gdstp,
            tc.tile_pool(name="redb", bufs=4) as redb,
            tc.tile_pool(name="redp", bufs=6) as redp,
            tc.tile_pool(name="xcp", bufs=1) as xcp,
            tc.tile_pool(name="hp", bufs=6) as hp,
            tc.tile_pool(name="ps_t", bufs=1, space="PSUM") as ps_t,
            tc.tile_pool(name="ps_h", bufs=3, space="PSUM") as ps_h,
            tc.tile_pool(name="ps_e", bufs=4, space="PSUM") as ps_e,
        ):
            # ---- bundled consts (tile now, DMA issued after the idx loads)
            cbt = constp.tile([128, 384 + 238], F16, name="cbt")
            ident = constp.tile([128, 128], F32, name="ident")
            make_identity(nc, ident[:])
            w1s = cbt[:, 0:256]
            w2a = cbt[:, 256:320]
            w2b = cbt[:, 320:384]
            cb32 = cbt[:, 384:622].bitcast(F32)
            wec = cb32[0:D, 0:64]
            bec = cb32[0:D, 64:65]
            rsq = cb32[:, 65:69]
            iaxc = cb32[0:D, 69:119]

            # on-device diag blocks: iax (f16), per-chunk rsq diag (f32)
            # (tiles allocated here; ops emitted after the cbt DMA below)
            iax = constp.tile([D, T_STEPS * D], F16, name="iax")
            rsqd = [constp.tile([128, 128], F32, name=f"rsqd{c}")
                    for c in range(NCHUNK)]

            diag_jobs = []

            def build_diags():
                for c in range(NCHUNK):
                    diag_jobs.append(lambda c=c: nc.vector.tensor_scalar(
                        out=rsqd[c][:], in0=ident[:], scalar1=rsq[:, c:c + 1],
                        scalar2=None, op0=mybir.AluOpType.mult))
                for t in range(T_STEPS):
                    # on ACT (idle in the gather window; DVE is reduce-bound)
                    diag_jobs.append(lambda t=t: nc.scalar.activation(
                        iax[:, t * D:(t + 1) * D], ident[0:D, 0:D],
                        mybir.ActivationFunctionType.Identity,
                        scale=iaxc[:, t:t + 1]))

            def emit_diags(n):
                while n > 0 and diag_jobs:
                    diag_jobs.pop(0)()
                    n -= 1

            nz = constp.tile([D, T_STEPS * BL], F16, name="nz")
            xout = constp.tile([D, BL], F16, name="xout")
            xcq = [xcp.tile([128, 128], F16, name=f"xc{q}", tag=f"xc{q}")
                   for q in range(NCHUNK)]
            poolT = [constp.tile([D, 128], F32, name=f"poolT{q}")
                     for q in range(NCHUNK)]

            idx_t = {}
            # per-chunk idx col offset for segment k (gather order = G desc)
            idx_off = {}
            for c in range(NCHUNK):
                ks = sorted(range(NSEG), key=lambda k: -G[c, k])
                off = 0
                for k in ks:
                    idx_off[(c, k)] = off
                    off += 8 * int(G[c, k])

            def load_idx(c, split_first=0):
                gs = int(G[c].sum())
                if split_first:
                    # first gather's idx in its own tile, loaded first
                    s = 8 * split_first
                    ita = gidxp.tile([128, s], I16, name=f"it{c}a", tag=f"it{c}a")
                    nc.sync.dma_start(ita[:], idx_d[c][:, 0:s])
                    it = gidxp.tile([128, 8 * gs - s], I16, name=f"it{c}",
                                    tag=f"it{c}")
                    nc.sync.dma_start(it[:], idx_d[c][:, s:])
                    idx_t[c] = (ita, it, s)
                else:
                    it = gidxp.tile([128, 8 * gs], I16, name=f"it{c}",
                                    tag=f"it{c}")
                    nc.sync.dma_start(it[:], idx_d[c][:])
                    idx_t[c] = (None, it, 0)

            def idx_ap(c, off, width):
                ita, it, s = idx_t[c]
                if ita is not None and off < s:
                    assert off + width <= s
                    return ita[:, off:off + width]
                return it[:, off - s:off - s + width]

            def do_gather(c, k, soff, g):
                off = idx_off[(c, k)] + 8 * soff
                dst = gdstp.tile([128, Gmax * D], FP8, name="dst", tag="dst")
                return dst, dma_gather_small(
                    nc.gpsimd,
                    dst[:, : g * D].rearrange("p (g d) -> p g d", g=g, d=D),
                    tbl_d[k * SEGR:(k + 1) * SEGR, 0:D],
                    idx_ap(c, off, 8 * g), 128 * g, 128 * g, D, 256)

            def do_reduce(g, dst, acc):
                """fp8 pair-add into bf16, bf16 tree to 2, mixed-add to f32."""
                ops = []
                m = g // 2
                if m == 0:
                    sk = redp.tile([128, D], F32, name="sk", tag="rk")
                    ops.append(nc.vector.tensor_copy(sk[:], dst[:, 0:D]))
                else:
                    red = redb.tile([128, (Gmax // 2 + 1) * D], BF16,
                                    name="red", tag="red")
                    op = nc.vector.tensor_tensor(
                        out=red[:, : m * D], in0=dst[:, : m * D],
                        in1=dst[:, m * D: 2 * m * D], op=mybir.AluOpType.add)
                    ops.append(op)
                    w = m
                    if g % 2:
                        ops.append(nc.vector.tensor_copy(
                            red[:, m * D:(m + 1) * D], dst[:, (g - 1) * D:g * D]))
                        w = m + 1
                    while w > 2:
                        mm2 = w // 2
                        ops.append(nc.vector.tensor_tensor(
                            out=red[:, : mm2 * D], in0=red[:, : mm2 * D],
                            in1=red[:, (w - mm2) * D: w * D],
                            op=mybir.AluOpType.add))
                        w = w - mm2
                    sk = redp.tile([128, D], F32, name="sk", tag="rk")
                    if w == 2:
                        ops.append(nc.vector.tensor_tensor(
                            out=sk[:], in0=red[:, 0:D], in1=red[:, D:2 * D],
                            op=mybir.AluOpType.add))
                    else:
                        ops.append(nc.vector.tensor_copy(sk[:], red[:, 0:D]))
                if acc is None:
                    return sk, ops
                acc2 = redp.tile([128, D], F32, name="acc2", tag="rk")
                ops.append(nc.vector.tensor_tensor(
                    out=acc2[:], in0=acc[:], in1=sk[:], op=mybir.AluOpType.add))
                return acc2, ops

            def do_finish_chunk(c, acc, on_dve=False):
                # transpose + rsq fold in one regular matmul:
                # pt = acc.T @ diag(rsq_c)
                pt = ps_t.tile([D, 128], F32, name="pt", tag="pt")
                nc.tensor.matmul(out=pt[:], lhsT=acc[:], rhs=rsqd[c][:],
                                 start=True, stop=True)
                # the LAST chunk's copy + bias-add go on DVE, not ACT: its
                # finish lands while ACT is busy with hoisted early silus,
                # but the DVE is idle right after its reduce drain and its
                # queue runs these ahead of the noise-gated step-0 updates.
                # Earlier chunks keep ACT (their finishes land mid-drain,
                # when DVE time is reduce time).
                if on_dve:
                    nc.vector.tensor_copy(poolT[c][:], pt[:])
                else:
                    nc.scalar.copy(poolT[c][:], pt[:])
                # conditioning for chain c
                pc = ps_t.tile([D, 128], F32, name="pc", tag="pt")
                nc.tensor.matmul(out=pc[:], lhsT=wec, rhs=poolT[c][:],
                                 start=True, stop=True)
                if on_dve:
                    nc.vector.tensor_scalar(
                        out=xcq[c][D:128, :], in0=pc[:], scalar1=bec,
                        scalar2=None, op0=mybir.AluOpType.add)
                else:
                    nc.scalar.activation(xcq[c][D:128, :], pc[:],
                                         mybir.ActivationFunctionType.Identity,
                                         bias=bec)

            # ---- phase 1: gathers + reduces, pipelined; within each chunk
            # the fattest segment first (leanest last => shortest tail).
            # The very first gather is split in two so its descriptor-gen
            # overlaps its own transfer.
            jobs = []
            for c in range(NCHUNK):
                ks = sorted(range(NSEG), key=lambda k: -G[c, k])
                jobs += [(c, k, 0, int(G[c, k])) for k in ks]
            # split the FIRST job: a small leading sub-gather primes the
            # DMA pipe earlier (short descgen before the first transfer)
            c0, k0, _, g0 = jobs[0]
            SPLIT0 = 12
            jobs[0:1] = [(c0, k0, 0, SPLIT0), (c0, k0, SPLIT0, g0 - SPLIT0)]
            # split the last job so the final reduce tail is shorter
            cl, kl, _, gl = jobs[-1]
            jobs[-1:] = [(cl, kl, 0, gl // 2), (cl, kl, gl // 2, gl - gl // 2)]
            left = {c: sum(1 for jb in jobs if jb[0] == c) for c in range(NCHUNK)}
            PIPE_G = 4
            load_idx(0, split_first=int(G[0, jobs[0][1]]))
            load_idx(1)
            nc.sync.dma_start(cbt[:], cb_d[:])
            build_diags()
            for q in range(NCHUNK):
                nc.sync.dma_start(xcq[q][0:D, :], x0T_d[:, q * 128:(q + 1) * 128])
            gdsts = {}
            for j in range(PIPE_G):
                gdsts[j] = do_gather(*jobs[j])
            accs = {c: None for c in range(NCHUNK)}
            warm_dep = None
            last_gather = None
            loaded = {0, 1}
            for j in range(len(jobs)):
                c, k, soff, g = jobs[j]
                if j + PIPE_G < len(jobs):
                    cn = jobs[j + PIPE_G][0]
                    if cn not in loaded:
                        load_idx(cn)
                        loaded.add(cn)
                    gdsts[j + PIPE_G] = do_gather(*jobs[j + PIPE_G])
                dst, ginst = gdsts.pop(j)
                if j == len(jobs) - 1:
                    last_gather = ginst
                accs[c], ops = do_reduce(g, dst, accs[c])
                if j == len(jobs) - 1:
                    final_red = ops[-1]
                emit_diags(4)
                if j == len(jobs) - 2:
                    warm_dep = ops[0]
                left[c] -= 1
                if left[c] == 0:
                    if j == len(jobs) - 1:
                        # PE warm-up BEFORE the last chunk-finish so it runs
                        # during the final gather/reduce, not after pc3 (the
                        # in-order PE queue would put it on the critical path)
                        warm_t = ps_t.tile([D, 128], F32, name="warm_t",
                                           tag="pt")
                        for i in range(N_WARM):
                            wm = nc.tensor.matmul(out=warm_t[:], lhsT=w2a,
                                                  rhs=w1s[:, 0:128],
                                                  start=True, stop=True)
                            if i == 0 and warm_dep is not None:
                                add_dep_helper(wm.ins, warm_dep.ins, sync=False,
                                               reason="warm near last reduce")
                    do_finish_chunk(c, accs[c],
                                    on_dve=(j == len(jobs) - 1))

            # ---- deferred DMAs (gated behind the last gather)
            def gated_dma(dst_ap, src_ap):
                inst = nc.sync.dma_start(dst_ap, src_ap)
                add_dep_helper(inst.ins, last_gather.ins, sync=True,
                               reason="defer until gathers done")
                return inst

            npc = T_STEPS // NZ_PIECES
            for p in range(NZ_PIECES):
                gated_dma(nz[:, p * npc * BL:(p + 1) * npc * BL],
                          noiseT_d[:, p * npc * BL:(p + 1) * npc * BL])

            # ---- phase 2: 50 steps, four 128-col chains in lockstep.
            # Matmuls grouped by stationary weight (5 LdWeights per wave);
            # noise+temb folded into the DVE x-update.
            for k in range(T_STEPS):
                live = [(q, k) for q in range(NCHUNK)]
                phs = {}
                for q, i in live:
                    phs[q] = ps_h.tile([128, 256], F32, name=f"ph{q}",
                                       tag="ph")
                for q, i in live:
                    nc.tensor.matmul(out=phs[q][:, 0:128], lhsT=w1s[:, 0:128],
                                     rhs=xcq[q][:], start=True, stop=True)
                for q, i in live:
                    nc.tensor.matmul(out=phs[q][:, 128:256],
                                     lhsT=w1s[:, 128:256],
                                     rhs=xcq[q][:], start=True, stop=True)
                hts = {}
                for q, i in live:
                    ht = hp.tile([128, 256], F16, name=f"h{q}", tag="h")
                    nc.scalar.activation(ht[:], phs[q][:],
                                         mybir.ActivationFunctionType.Silu)
                    hts[q] = ht
                pes = {}
                for q, i in live:
                    pes[q] = ps_e.tile([D, 128], F32, name=f"pe{q}", tag="pe")
                for q, i in live:
                    t = T_STEPS - 1 - i
                    nc.tensor.matmul(out=pes[q][:],
                                     lhsT=iax[:, t * D:(t + 1) * D],
                                     rhs=xcq[q][0:D, :], start=True, stop=False)
                for q, i in live:
                    nc.tensor.matmul(out=pes[q][:], lhsT=w2a,
                                     rhs=hts[q][:, 0:128],
                                     start=False, stop=False)
                for q, i in live:
                    nc.tensor.matmul(out=pes[q][:], lhsT=w2b,
                                     rhs=hts[q][:, 128:256],
                                     start=False, stop=True)
                for q, i in live:
                    t = T_STEPS - 1 - i
                    col = i * BL + q * 128
                    dst = (xcq[q][0:D, :] if i < T_STEPS - 1
                           else xout[:, q * 128:(q + 1) * 128])
                    stt = nc.vector.scalar_tensor_tensor(
                        out=dst, in0=pes[q][:],
                        scalar=-float(C[t]), in1=nz[:, col:col + 128],
                        op0=mybir.AluOpType.mult, op1=mybir.AluOpType.add)
                    if i == 0:
                        # keep the step-0 x-updates BEHIND the final reduce
                        # in the DVE queue: they stall on the noise load, and
                        # scheduled ahead they head-block the last chunk's
                        # reduce -> conditioning -> the whole critical chain
                        add_dep_helper(stt.ins, final_red.ins, sync=False,
                                       reason="step-0 upd after final reduce")

            nc.sync.dma_start(outT_d[:], xout[:])

    nc.compile()
    return nc


_CACHE = {}


def _get_program(G, consts):
    key = tuple(G.reshape(-1).tolist())
    if key not in _CACHE:
        _CACHE[key] = build_program(G, consts)
    return _CACHE[key]


def kernel(**inputs):
    per_core, G, consts = host_prep(inputs)
    nc = _get_program(G, consts)
    in_maps = [core for core, _ in per_core]
    res = run_bass_kernel_spmd(nc, in_maps, list(range(NCORES)))
    out = np.zeros((B, D), np.float32)
    for n in range(NCORES):
        _, rws = per_core[n]
        out[rws] = np.asarray(res.results[n]["outT"]).astype(np.float32).T
    return out

